# revision 1
# baseline (speedup 1.0000x reference)
"""Multi-head attention (B=4, S=2048, D=1024, H=16) on 8 trn2 NeuronCores.

Sharding: batch x query-half. Core c handles batch c//2, query rows
(c%2)*1024 : (c%2+1)*1024. Each core projects Q for its query chunk and K/V
for the full sequence of its batch (K/V projection duplicated across the two
cores sharing a batch), runs attention for all 16 heads, and applies the
output projection. No cross-core communication.

Device-side layout notes:
 - All activations are kept transposed ([feature, token]) so every matmul
   consumes operands directly: scores are computed as S^T[k,q] = K_h^T.T @ Q_h^T,
   softmax-exp runs on ScalarE, and the AV matmul contracts over k with
   lhsT = [V_h | ones-column] (stride 65), which makes row 64 of the PSUM
   output the softmax denominator. Normalization: DVE reciprocal + a 1x64
   PE matmul to replicate it across partitions + DVE multiply. No max
   subtraction (scores are O(+-5) here, fp32 exp is safe).
 - Head pairs (2h, 2h+1) sit in partitions 0:64 / 64:128 of the same tile;
   their QK matmuls use disjoint PE row-groups and run concurrently.
 - Both heads' scores land in one [128,1024] PSUM tile -> a single ScalarE
   exp instruction, halving ACT instruction overhead.
 - Matmul operands are bf16; accumulation is fp32 in PSUM.
 - Phase order: K proj, Q proj, then attention with the V projection fused
   into the first head-pair's loop and the qc0 output projection interleaved
   into qc1's attention — keeps TensorE fed while ScalarE runs exp.
 - V bias is folded into the output-projection bias host-side
   (softmax rows sum to 1 => attn @ (V + 1 b_v^T) = attn @ V + b_v^T).
"""

import numpy as np

B, S, D, H = 4, 2048, 1024, 16
DK = D // H          # 64
TQ = S // 2          # per-core query tokens
TK = S               # per-core key tokens
CW = 512             # x^T streaming chunk width (tokens)
N_CORES = 8
VP_W = H * (DK + 1)  # per head: 64 V columns + 1 ones column (stride 65)
SCALE = 1.0 / np.sqrt(DK)

_CACHE = {}


def _build_program(reps=1):
    import concourse.bass as bass
    import concourse.mybir as mybir
    from concourse import bacc
    from concourse.tile import TileContext

    f32 = mybir.dt.float32
    bf16 = mybir.dt.bfloat16
    AF = mybir.ActivationFunctionType

    nc = bacc.Bacc("TRN2", target_bir_lowering=False)

    xqT = nc.declare_dram_parameter("xqT", [D, TQ], bf16, isOutput=False)
    xkT = nc.declare_dram_parameter("xkT", [D, TK], bf16, isOutput=False)
    xvT = nc.declare_dram_parameter("xvT", [D, TK], bf16, isOutput=False)
    wqT = nc.declare_dram_parameter("wqT", [D, D], bf16, isOutput=False)
    wkT = nc.declare_dram_parameter("wkT", [D, D], bf16, isOutput=False)
    wvT = nc.declare_dram_parameter("wvT", [D, D], bf16, isOutput=False)
    woT = nc.declare_dram_parameter("woT", [D, D], bf16, isOutput=False)
    bq_in = nc.declare_dram_parameter("bq_in", [128, 8], f32, isOutput=False)
    bk_in = nc.declare_dram_parameter("bk_in", [128, 8], f32, isOutput=False)
    bo_in = nc.declare_dram_parameter("bo_in", [128, 8], f32, isOutput=False)
    yT = nc.declare_dram_parameter("yT", [D, TQ], f32, isOutput=True)

    # DRAM access helpers: feature dim split as (tile j, partition p)
    xq_r = xqT[:].rearrange("(a p) t -> p a t", p=128)
    xk_r = xkT[:].rearrange("(a p) t -> p a t", p=128)
    xv_r = xvT[:].rearrange("(a p) t -> p a t", p=128)
    wq_r = wqT[:].rearrange("(a p) d -> p a d", p=128)
    wk_r = wkT[:].rearrange("(a p) d -> p a d", p=128)
    wv_r = wvT[:].rearrange("(a p) d -> p a d", p=128)
    wo_r = woT[:].rearrange("(a p) d -> p a d", p=128)

    with TileContext(nc) as tc:
        for _rep in range(reps):
            _emit_body(nc, tc, bass, f32, bf16, AF,
                       xq_r, xk_r, xv_r, wq_r, wk_r, wv_r, wo_r,
                       bq_in, bk_in, bo_in, yT)
    nc.compile()
    return nc


def _emit_body(nc, tc, bass, f32, bf16, AF,
               xq_r, xk_r, xv_r, wq_r, wk_r, wv_r, wo_r,
               bq_in, bk_in, bo_in, yT):
    def mm(out, lhsT, rhs, start, stop):
        nc.tensor.matmul(out, lhsT=lhsT, rhs=rhs, start=start, stop=stop)

    if True:
        with (
            tc.tile_pool(name="const", bufs=1) as const_pool,
            tc.tile_pool(name="kt_res", bufs=1) as kt_pool,
            tc.tile_pool(name="qt_res", bufs=1) as qt_pool,
            tc.tile_pool(name="vp_res", bufs=1) as vp_pool,
            tc.tile_pool(name="ot_res", bufs=2) as ot_pool,
            tc.tile_pool(name="w_res", bufs=3) as w_pool,
            tc.tile_pool(name="x_str", bufs=3) as x_pool,
            tc.tile_pool(name="exp_p", bufs=4) as exp_pool,
            tc.tile_pool(name="rec_p", bufs=2) as rec_pool,
            tc.tile_pool(name="recb_p", bufs=3) as recb_pool,
            tc.tile_pool(name="oc_p", bufs=6) as oc_pool,
            tc.tile_pool(name="y_p", bufs=4) as y_pool,
            tc.tile_pool(name="ps_proj", bufs=2, space="PSUM") as ps_proj,
            tc.tile_pool(name="ps_s", bufs=2, space="PSUM") as ps_s,
            tc.tile_pool(name="ps_av", bufs=2, space="PSUM") as ps_av,
        ):
            bq_sb = const_pool.tile([128, 8], f32, tag="bq")
            bk_sb = const_pool.tile([128, 8], f32, tag="bk")
            bo_sb = const_pool.tile([128, 8], f32, tag="bo")
            # separate queue: keep these tiny loads off the head of the
            # sync queue that feeds the first matmuls
            nc.gpsimd.dma_start(out=bq_sb, in_=bq_in[:])
            nc.gpsimd.dma_start(out=bk_sb, in_=bk_in[:])
            nc.gpsimd.dma_start(out=bo_sb, in_=bo_in[:])

            KT_sb = kt_pool.tile([128, 8, TK], bf16, tag="KT")    # [p, j, t]
            QT_sb = qt_pool.tile([128, 8, TQ], bf16, tag="QT")    # [p, j, t]
            Vp_sb = vp_pool.tile([128, 16, VP_W], bf16, tag="Vp")  # [p, i, c]
            # view: [p, ktile, head, col(65)]
            Vp4 = Vp_sb.rearrange("p i (hh c) -> p i hh c", c=DK + 1)
            nc.vector.memset(Vp4[:, :, :, DK], 1.0)
            ones_sb = const_pool.tile([1, 64], bf16, tag="ones")
            nc.vector.memset(ones_sb, 1.0)

            # ---- K projection: K^T[dout, t], streamed x chunks ----
            # split loads so the first dj-column's matmuls start early
            wk_sb = w_pool.tile([128, 8, D], bf16, tag="wbig", name="wk_sb")
            nc.sync.dma_start(out=wk_sb[:, :, 0:128], in_=wk_r[:, :, 0:128])
            for tci in range(TK // CW):
                xc = x_pool.tile([128, 8, CW], bf16, tag="xchunk",
                                 name=f"xk_{tci}")
                if tci == 0:
                    for kq in range(4):
                        nc.sync.dma_start(
                            out=xc[:, 2 * kq:2 * kq + 2, :],
                            in_=xk_r[:, 2 * kq:2 * kq + 2, 0:CW])
                    # remaining K-weight columns, one dj-slice each, queued
                    # behind the first activation chunk
                    for djw in range(1, 8):
                        nc.sync.dma_start(
                            out=wk_sb[:, :, djw * 128:(djw + 1) * 128],
                            in_=wk_r[:, :, djw * 128:(djw + 1) * 128])
                else:
                    nc.sync.dma_start(out=xc,
                                      in_=xk_r[:, :, tci * CW:(tci + 1) * CW])
                for dj in range(8):
                    ps = ps_proj.tile([128, CW], f32, tag="pp",
                                      name=f"pk_{tci}_{dj}")
                    for kj in range(8):
                        mm(ps, wk_sb[:, kj, dj * 128:(dj + 1) * 128],
                           xc[:, kj, :], kj == 0, kj == 7)
                    nc.vector.tensor_scalar_add(
                        out=KT_sb[:, dj, tci * CW:(tci + 1) * CW],
                        in0=ps, scalar1=bk_sb[:, dj:dj + 1])

            # ---- Q projection ----
            wq_sb = w_pool.tile([128, 8, D], bf16, tag="wbig", name="wq_sb")
            nc.sync.dma_start(out=wq_sb, in_=wq_r)

            xq_tiles = {}

            def q_proj_dj(tci, dj):
                if dj == 0:
                    xq_tiles[tci] = x_pool.tile([128, 8, CW], bf16,
                                                tag="xchunk",
                                                name=f"xq_{tci}")
                    nc.sync.dma_start(
                        out=xq_tiles[tci],
                        in_=xq_r[:, :, tci * CW:(tci + 1) * CW])
                xc = xq_tiles[tci]
                ps = ps_proj.tile([128, CW], f32, tag="pp",
                                  name=f"pq_{tci}_{dj}")
                for kj in range(8):
                    mm(ps, wq_sb[:, kj, dj * 128:(dj + 1) * 128],
                       xc[:, kj, :], kj == 0, kj == 7)
                nc.vector.tensor_scalar_add(
                    out=QT_sb[:, dj, tci * CW:(tci + 1) * CW],
                    in0=ps, scalar1=bq_sb[:, dj:dj + 1])

            def q_proj_chunk(tci):
                for dj in range(8):
                    q_proj_dj(tci, dj)

            # V and Wo weights resident; V projection is fused into the first
            # head-pair's attention loop below so ScalarE exp overlaps it
            wv_sb = w_pool.tile([128, 8, D], bf16, tag="wbig", name="wv_sb")
            nc.sync.dma_start(out=wv_sb, in_=wv_r)
            wo_sb = w_pool.tile([128, 8, D], bf16, tag="wbig", name="wo_sb")
            nc.sync.dma_start(out=wo_sb, in_=wo_r)

            def v_proj_chunk(tci):
                xc = x_pool.tile([128, 8, CW], bf16, tag="xchunk",
                                 name=f"xv_{tci}")
                nc.sync.dma_start(out=xc,
                                  in_=xv_r[:, :, tci * CW:(tci + 1) * CW])
                for ts2 in range(CW // 128):
                    ti = tci * (CW // 128) + ts2
                    for dc in range(2):
                        ps = ps_proj.tile([128, CW], f32, tag="pp",
                                          name=f"pv_{ti}_{dc}")
                        for kj in range(8):
                            mm(ps[:, 0:512], xc[:, kj, ts2 * 128:(ts2 + 1) * 128],
                               wv_sb[:, kj, dc * 512:(dc + 1) * 512],
                               kj == 0, kj == 7)
                        nc.vector.tensor_copy(
                            out=Vp4[:, ti, dc * 8:(dc + 1) * 8, 0:DK],
                            in_=ps[:, 0:512].rearrange("p (hh c) -> p hh c", c=DK))

            OT_tiles = {}

            # deferred normalization: (qc, hp, sbuf copy of [65,512] accum)
            pending_norm = []

            def flush_norm():
                while pending_norm:
                    qc, hp, oc = pending_norm.pop(0)
                    for hh in range(2):
                        # row 64 of oc = softmax denominator
                        rec = rec_pool.tile([1, 512], bf16, tag="rec",
                                            name=f"rec_{qc}_{hp}_{hh}")
                        with nc.allow_low_precision(
                                reason="softmax denom reciprocal, bf16 "
                                       "matches pipeline precision"):
                            nc.vector.reciprocal(out=rec,
                                                 in_=oc[hh][64:65, :])
                        # replicate reciprocal across 64 partitions via PE
                        ps_rep = ps_proj.tile([128, CW], f32, tag="pp",
                                              name=f"pr_{qc}_{hp}_{hh}")
                        mm(ps_rep[0:64, 0:512], ones_sb, rec, True, True)
                        recb = recb_pool.tile([64, 512], f32, tag="recb",
                                              name=f"recb_{qc}_{hp}_{hh}")
                        nc.vector.tensor_copy(out=recb,
                                              in_=ps_rep[0:64, 0:512])
                        nc.vector.tensor_mul(
                            out=OT_tiles[qc][hh * 64:(hh + 1) * 64, hp, :],
                            in0=oc[hh][0:64, :], in1=recb)

            def attn_hp(qc, hp, fuse_v=False, fuse_k=None, fill=None):
                qsl = slice(qc * 512, (qc + 1) * 512)
                ps_o = [ps_av.tile([128, 512], f32, tag="po",
                                   name=f"po_{qc}_{hp}_{i}")
                        for i in range(2)]
                def qk_exp(kt):
                    # both heads' scores^T into one 2-bank PSUM tile
                    pss = ps_s.tile([128, 1024], f32, tag="pss",
                                    name=f"pss_{qc}_{hp}_{kt}")
                    for hh in range(2):
                        pb = hh * 64
                        mm(pss[:, hh * 512:(hh + 1) * 512],
                           KT_sb[pb:pb + 64, hp, kt * 128:(kt + 1) * 128],
                           QT_sb[pb:pb + 64, hp, qsl], True, True)
                    e = exp_pool.tile([128, 1024], bf16, tag="ex",
                                      name=f"ex_{qc}_{hp}_{kt}")
                    nc.scalar.activation(out=e, in_=pss, func=AF.Exp,
                                         scale=SCALE)
                    return e

                def av(kt, e):
                    for hh in range(2):
                        h = 2 * hp + hh
                        mm(ps_o[hh][0:65, :],
                           Vp_sb[:, kt, 65 * h:65 * h + 65],
                           e[:, hh * 512:(hh + 1) * 512],
                           kt == 0, kt == 15)

                # QK/exp run one kt ahead of AV so the pair-boundary
                # accumulator release is off the PE critical path
                e_prev = None
                for kt in range(16):
                    if fuse_v and kt % 4 == 0:
                        v_proj_chunk(kt // 4)
                    if fuse_k is not None and kt % 4 == 0:
                        k_proj_tci(fuse_k, kt // 4)
                    if fill and kt % 8 == 4:
                        fill.pop(0)()   # PE fill-in during ACT-bound stretch
                    if kt == 2:
                        # previous pair's normalization, now off the
                        # critical path (its PE replicate slots in here)
                        flush_norm()
                    e = qk_exp(kt)
                    if e_prev is not None:
                        av(kt - 1, e_prev)
                    e_prev = e
                av(15, e_prev)
                # copy accumulators (incl. denominator row) to SBUF right
                # away: frees both PSUM slots for the next pair's AVs
                oc = []
                for hh in range(2):
                    o_sb = oc_pool.tile([65, 512], f32, tag="oc",
                                        name=f"oc_{qc}_{hp}_{hh}")
                    nc.vector.tensor_copy(out=o_sb, in_=ps_o[hh][0:65, :])
                    oc.append(o_sb)
                pending_norm.append((qc, hp, oc))

            def wo_dj(qc, dj):
                qsl = slice(qc * 512, (qc + 1) * 512)
                ps_y = ps_proj.tile([128, CW], f32, tag="pp",
                                    name=f"py_{qc}_{dj}")
                for kj in range(8):
                    mm(ps_y[:, 0:512], wo_sb[:, kj, dj * 128:(dj + 1) * 128],
                       OT_tiles[qc][:, kj, :], kj == 0, kj == 7)
                yt = y_pool.tile([128, 512], f32, tag="yt",
                                 name=f"yt_{qc}_{dj}")
                nc.vector.tensor_scalar_add(
                    out=yt, in0=ps_y[:, 0:512], scalar1=bo_sb[:, dj:dj + 1])
                nc.sync.dma_start(
                    out=yT[dj * 128:(dj + 1) * 128, qsl], in_=yt)

            OT_tiles[0] = ot_pool.tile([128, 8, 512], bf16, tag="OT",
                                       name="OT_0")
            q_proj_chunk(0)        # QT for query-chunk 0
            # QT chunk 1 is produced as PE fill-in inside qc0's ACT-bound
            # head-pair loops (one dj-block per slot, hp1..hp4)
            fill_q = [lambda tci=1, dj=dj: q_proj_dj(tci, dj)
                      for dj in range(8)]
            for hp in range(8):
                fills = fill_q[2 * (hp - 1):2 * hp] if 1 <= hp <= 4 else None
                attn_hp(0, hp, fuse_v=(hp == 0), fill=fills)
            OT_tiles[1] = ot_pool.tile([128, 8, 512], bf16, tag="OT",
                                       name="OT_1")
            for hp in range(8):
                attn_hp(1, hp)
                if hp == 7:
                    # last pair's normalization before the final Wo block so
                    # its DVE chain hides under wo_dj(0,7)'s matmuls
                    flush_norm()
                wo_dj(0, hp)       # overlap qc0 output proj with qc1 attention
            for dj in range(8):
                wo_dj(1, dj)


def _prep_inputs(query, key, value, Wq, bq, Wk, bk, Wv, bv, Wo, bo):
    import ml_dtypes
    bf = ml_dtypes.bfloat16

    query = np.asarray(query, np.float32)
    key = np.asarray(key, np.float32)
    value = np.asarray(value, np.float32)
    wqT = np.ascontiguousarray(np.asarray(Wq, np.float32).T.astype(bf))
    wkT = np.ascontiguousarray(np.asarray(Wk, np.float32).T.astype(bf))
    wvT = np.ascontiguousarray(np.asarray(Wv, np.float32).T.astype(bf))
    woT = np.ascontiguousarray(np.asarray(Wo, np.float32).T.astype(bf))
    bo_eff = np.asarray(bo, np.float32) + \
        np.asarray(Wo, np.float32) @ np.asarray(bv, np.float32)
    bq_t = np.ascontiguousarray(np.asarray(bq, np.float32).reshape(8, 128).T)
    bk_t = np.ascontiguousarray(np.asarray(bk, np.float32).reshape(8, 128).T)
    bo_t = np.ascontiguousarray(bo_eff.reshape(8, 128).T)

    in_maps = []
    for c in range(N_CORES):
        b, qh = c // 2, c % 2
        in_maps.append({
            "xqT": np.ascontiguousarray(
                query[b, qh * TQ:(qh + 1) * TQ, :].T.astype(bf)),
            "xkT": np.ascontiguousarray(key[b].T.astype(bf)),
            "xvT": np.ascontiguousarray(value[b].T.astype(bf)),
            "wqT": wqT, "wkT": wkT, "wvT": wvT, "woT": woT,
            "bq_in": bq_t, "bk_in": bk_t, "bo_in": bo_t,
        })
    return in_maps


def kernel(query, key, value, Wq, bq, Wk, bk, Wv, bv, Wo, bo):
    from concourse.bass_utils import run_bass_kernel_spmd

    if "nc" not in _CACHE:
        _CACHE["nc"] = _build_program()
    nc = _CACHE["nc"]

    in_maps = _prep_inputs(query, key, value, Wq, bq, Wk, bk, Wv, bv, Wo, bo)
    res = run_bass_kernel_spmd(nc, in_maps, list(range(N_CORES)))
    out = np.empty((B, S, D), np.float32)
    for c in range(N_CORES):
        b, qh = c // 2, c % 2
        out[b, qh * TQ:(qh + 1) * TQ, :] = res.results[c]["yT"].T
    return out



# revision 6
# speedup vs baseline: 1.0535x; 1.0535x over previous
"""Multi-head attention (B=4, S=2048, D=1024, H=16) on 8 trn2 NeuronCores.

Sharding: batch x query-half. Core c handles batch c//2, query rows
(c%2)*1024 : (c%2+1)*1024. Each core projects Q for its query chunk and K/V
for the full sequence of its batch (K/V projection duplicated across the two
cores sharing a batch), runs attention for all 16 heads, and applies the
output projection. No cross-core communication.

Device-side layout notes:
 - All activations are kept transposed ([feature, token]) so every matmul
   consumes operands directly: scores are computed as S^T[k,q] = K_h^T.T @ Q_h^T,
   softmax-exp runs on ScalarE, and the AV matmul contracts over k with
   lhsT = [V_h | ones-column] (stride 65), which makes row 64 of the PSUM
   output the softmax denominator. Normalization: DVE reciprocal + a 1x64
   PE matmul to replicate it across partitions + DVE multiply. No max
   subtraction (scores are O(+-5) here, fp32 exp is safe).
 - Head pairs (2h, 2h+1) sit in partitions 0:64 / 64:128 of the same tile;
   their QK matmuls use disjoint PE row-groups and run concurrently.
 - Both heads' scores land in one [128,1024] PSUM tile -> a single ScalarE
   exp instruction, halving ACT instruction overhead.
 - Matmul operands are bf16; accumulation is fp32 in PSUM.
 - Phase order: K proj, Q proj, then attention with the V projection fused
   into the first head-pair's loop and the qc0 output projection interleaved
   into qc1's attention — keeps TensorE fed while ScalarE runs exp.
 - V bias is folded into the output-projection bias host-side
   (softmax rows sum to 1 => attn @ (V + 1 b_v^T) = attn @ V + b_v^T).
"""

import numpy as np

B, S, D, H = 4, 2048, 1024, 16
DK = D // H          # 64
TQ = S // 2          # per-core query tokens
TK = S               # per-core key tokens
CW = 512             # x^T streaming chunk width (tokens)
N_CORES = 8
VP_W = H * (DK + 1)  # per head: 64 V columns + 1 ones column (stride 65)
SCALE = 1.0 / np.sqrt(DK)

_CACHE = {}


def _build_program(reps=1):
    import concourse.bass as bass
    import concourse.mybir as mybir
    from concourse import bacc
    from concourse.tile import TileContext

    f32 = mybir.dt.float32
    bf16 = mybir.dt.bfloat16
    AF = mybir.ActivationFunctionType

    nc = bacc.Bacc("TRN2", target_bir_lowering=False)

    xqT = nc.declare_dram_parameter("xqT", [D, TQ], bf16, isOutput=False)
    xkT = nc.declare_dram_parameter("xkT", [D, TK], bf16, isOutput=False)
    xvT = nc.declare_dram_parameter("xvT", [D, TK], bf16, isOutput=False)
    wqT = nc.declare_dram_parameter("wqT", [D, D], bf16, isOutput=False)
    wkT = nc.declare_dram_parameter("wkT", [D, D], bf16, isOutput=False)
    wvT = nc.declare_dram_parameter("wvT", [D, D], bf16, isOutput=False)
    woT = nc.declare_dram_parameter("woT", [D, D], bf16, isOutput=False)
    bq_in = nc.declare_dram_parameter("bq_in", [128, 8], f32, isOutput=False)
    bk_in = nc.declare_dram_parameter("bk_in", [128, 8], f32, isOutput=False)
    bo_in = nc.declare_dram_parameter("bo_in", [128, 8], f32, isOutput=False)
    yT = nc.declare_dram_parameter("yT", [D, TQ], f32, isOutput=True)

    # DRAM access helpers: feature dim split as (tile j, partition p)
    xq_r = xqT[:].rearrange("(a p) t -> p a t", p=128)
    xk_r = xkT[:].rearrange("(a p) t -> p a t", p=128)
    xv_r = xvT[:].rearrange("(a p) t -> p a t", p=128)
    wq_r = wqT[:].rearrange("(a p) d -> p a d", p=128)
    wk_r = wkT[:].rearrange("(a p) d -> p a d", p=128)
    wv_r = wvT[:].rearrange("(a p) d -> p a d", p=128)
    wo_r = woT[:].rearrange("(a p) d -> p a d", p=128)

    with TileContext(nc) as tc:
        for _rep in range(reps):
            _emit_body(nc, tc, bass, f32, bf16, AF,
                       xq_r, xk_r, xv_r, wq_r, wk_r, wv_r, wo_r,
                       bq_in, bk_in, bo_in, yT)
    nc.compile()
    return nc


def _emit_body(nc, tc, bass, f32, bf16, AF,
               xq_r, xk_r, xv_r, wq_r, wk_r, wv_r, wo_r,
               bq_in, bk_in, bo_in, yT):
    import concourse.mybir as mybir
    def mm(out, lhsT, rhs, start, stop):
        nc.tensor.matmul(out, lhsT=lhsT, rhs=rhs, start=start, stop=stop)

    if True:
        with (
            tc.tile_pool(name="const", bufs=1) as const_pool,
            tc.tile_pool(name="kt_res", bufs=1) as kt_pool,
            tc.tile_pool(name="qt_res", bufs=1) as qt_pool,
            tc.tile_pool(name="vp_res", bufs=1) as vp_pool,
            tc.tile_pool(name="ot_res", bufs=2) as ot_pool,
            tc.tile_pool(name="w_res", bufs=3) as w_pool,
            tc.tile_pool(name="x_str", bufs=3) as x_pool,
            tc.tile_pool(name="exp_p", bufs=4) as exp_pool,
            tc.tile_pool(name="rec_p", bufs=2) as rec_pool,
            tc.tile_pool(name="recb_p", bufs=3) as recb_pool,
            tc.tile_pool(name="oc_p", bufs=6) as oc_pool,
            tc.tile_pool(name="y_p", bufs=4) as y_pool,
            tc.tile_pool(name="ps_proj", bufs=2, space="PSUM") as ps_proj,
            tc.tile_pool(name="ps_s", bufs=2, space="PSUM") as ps_s,
            tc.tile_pool(name="ps_av", bufs=2, space="PSUM") as ps_av,
        ):
            bq_sb = const_pool.tile([128, 8], f32, tag="bq")
            bk_sb = const_pool.tile([128, 8], f32, tag="bk")
            bo_sb = const_pool.tile([128, 8], f32, tag="bo")
            # separate queue: keep these tiny loads off the head of the
            # sync queue that feeds the first matmuls
            nc.gpsimd.dma_start(out=bq_sb, in_=bq_in[:])
            nc.gpsimd.dma_start(out=bk_sb, in_=bk_in[:])
            nc.gpsimd.dma_start(out=bo_sb, in_=bo_in[:])

            fp8 = mybir.dt.float8e4
            # [p, hp, j, t]: j=0 -> K_hi / Q8, j=1 -> K_lo residual / Q8 dup
            KT_sb = kt_pool.tile([128, 8, 2, TK], fp8, tag="KT")
            QT_sb = qt_pool.tile([128, 8, 2, TQ], fp8, tag="QT")
            Vp_sb = vp_pool.tile([128, 16, VP_W], bf16, tag="Vp")  # [p, i, c]
            # view: [p, ktile, head, col(65)]
            Vp4 = Vp_sb.rearrange("p i (hh c) -> p i hh c", c=DK + 1)
            nc.vector.memset(Vp4[:, :, :, DK], 1.0)
            ones_sb = const_pool.tile([1, 64], bf16, tag="ones")
            nc.vector.memset(ones_sb, 1.0)

            # ---- K projection: K^T[dout, t], streamed x chunks ----
            # split loads so the first dj-column's matmuls start early
            wk_sb = w_pool.tile([128, 8, D], bf16, tag="wbig", name="wk_sb")
            nc.sync.dma_start(out=wk_sb[:, :, 0:128], in_=wk_r[:, :, 0:128])
            for tci in range(TK // CW):
                xc = x_pool.tile([128, 8, CW], bf16, tag="xchunk",
                                 name=f"xk_{tci}")
                if tci == 0:
                    for kq in range(4):
                        nc.sync.dma_start(
                            out=xc[:, 2 * kq:2 * kq + 2, :],
                            in_=xk_r[:, 2 * kq:2 * kq + 2, 0:CW])
                    # remaining K-weight columns, one dj-slice each, queued
                    # behind the first activation chunk
                    for djw in range(1, 8):
                        nc.sync.dma_start(
                            out=wk_sb[:, :, djw * 128:(djw + 1) * 128],
                            in_=wk_r[:, :, djw * 128:(djw + 1) * 128])
                else:
                    nc.sync.dma_start(out=xc,
                                      in_=xk_r[:, :, tci * CW:(tci + 1) * CW])
                for dj in range(8):
                    ps = ps_proj.tile([128, CW], f32, tag="pp",
                                      name=f"pk_{tci}_{dj}")
                    for kj in range(8):
                        mm(ps, wk_sb[:, kj, dj * 128:(dj + 1) * 128],
                           xc[:, kj, :], kj == 0, kj == 7)
                    # bk is dropped: softmax over keys is invariant to the
                    # (Q+bq)@bk term (constant per query). K_hi = fp8(K),
                    # K_lo = fp8(K - K_hi) -> DoubleRow pair corrects the
                    # K-side quantization error.
                    ksl = slice(tci * CW, (tci + 1) * CW)
                    nc.vector.tensor_copy(out=KT_sb[:, dj, 0, ksl], in_=ps)
                    with nc.allow_low_precision(
                            reason="fp8 residual capture for DoubleRow QK"):
                        nc.vector.tensor_sub(out=KT_sb[:, dj, 1, ksl],
                                             in0=ps, in1=KT_sb[:, dj, 0, ksl])

            # ---- Q projection ----
            wq_sb = w_pool.tile([128, 8, D], bf16, tag="wbig", name="wq_sb")
            nc.sync.dma_start(out=wq_sb, in_=wq_r)

            xq_tiles = {}

            def q_proj_dj(tci, dj):
                if dj == 0:
                    xq_tiles[tci] = x_pool.tile([128, 8, CW], bf16,
                                                tag="xchunk",
                                                name=f"xq_{tci}")
                    nc.sync.dma_start(
                        out=xq_tiles[tci],
                        in_=xq_r[:, :, tci * CW:(tci + 1) * CW])
                xc = xq_tiles[tci]
                ps = ps_proj.tile([128, CW], f32, tag="pp",
                                  name=f"pq_{tci}_{dj}")
                for kj in range(8):
                    mm(ps, wq_sb[:, kj, dj * 128:(dj + 1) * 128],
                       xc[:, kj, :], kj == 0, kj == 7)
                qsl = slice(tci * CW, (tci + 1) * CW)
                nc.vector.tensor_scalar_add(
                    out=QT_sb[:, dj, 0, qsl], in0=ps,
                    scalar1=bq_sb[:, dj:dj + 1])
                # duplicate Q8 into the second DoubleRow slot (pairs with
                # K_lo); cheap 16-bit-view copy
                nc.vector.tensor_copy(
                    out=QT_sb[:, dj, 1, qsl].bitcast(mybir.dt.uint16),
                    in_=QT_sb[:, dj, 0, qsl].bitcast(mybir.dt.uint16))

            def q_proj_chunk(tci):
                for dj in range(8):
                    q_proj_dj(tci, dj)

            # V and Wo weights resident; V projection is fused into the first
            # head-pair's attention loop below so ScalarE exp overlaps it
            wv_sb = w_pool.tile([128, 8, D], bf16, tag="wbig", name="wv_sb")
            nc.sync.dma_start(out=wv_sb, in_=wv_r)
            wo_sb = w_pool.tile([128, 8, D], bf16, tag="wbig", name="wo_sb")
            nc.sync.dma_start(out=wo_sb, in_=wo_r)

            def v_proj_chunk(tci):
                xc = x_pool.tile([128, 8, CW], bf16, tag="xchunk",
                                 name=f"xv_{tci}")
                nc.sync.dma_start(out=xc,
                                  in_=xv_r[:, :, tci * CW:(tci + 1) * CW])
                for ts2 in range(CW // 128):
                    ti = tci * (CW // 128) + ts2
                    for dc in range(2):
                        ps = ps_proj.tile([128, CW], f32, tag="pp",
                                          name=f"pv_{ti}_{dc}")
                        for kj in range(8):
                            mm(ps[:, 0:512], xc[:, kj, ts2 * 128:(ts2 + 1) * 128],
                               wv_sb[:, kj, dc * 512:(dc + 1) * 512],
                               kj == 0, kj == 7)
                        nc.vector.tensor_copy(
                            out=Vp4[:, ti, dc * 8:(dc + 1) * 8, 0:DK],
                            in_=ps[:, 0:512].rearrange("p (hh c) -> p hh c", c=DK))

            OT_tiles = {}

            # deferred normalization: (qc, hp, sbuf copy of [65,512] accum)
            pending_norm = []

            def flush_norm():
                while pending_norm:
                    qc, hp, oc = pending_norm.pop(0)
                    for hh in range(2):
                        # row 64 of oc = softmax denominator
                        rec = rec_pool.tile([1, 512], bf16, tag="rec",
                                            name=f"rec_{qc}_{hp}_{hh}")
                        with nc.allow_low_precision(
                                reason="softmax denom reciprocal, bf16 "
                                       "matches pipeline precision"):
                            nc.vector.reciprocal(out=rec,
                                                 in_=oc[hh][64:65, :])
                        # replicate reciprocal across 64 partitions via PE
                        ps_rep = ps_proj.tile([128, CW], f32, tag="pp",
                                              name=f"pr_{qc}_{hp}_{hh}")
                        mm(ps_rep[0:64, 0:512], ones_sb, rec, True, True)
                        recb = recb_pool.tile([64, 512], f32, tag="recb",
                                              name=f"recb_{qc}_{hp}_{hh}")
                        nc.vector.tensor_copy(out=recb,
                                              in_=ps_rep[0:64, 0:512])
                        nc.vector.tensor_mul(
                            out=OT_tiles[qc][hh * 64:(hh + 1) * 64, hp, :],
                            in0=oc[hh][0:64, :], in1=recb)

            def attn_hp(qc, hp, fuse_v=False, fuse_k=None, fill=None):
                qsl = slice(qc * 512, (qc + 1) * 512)
                ps_o = [ps_av.tile([128, 512], f32, tag="po",
                                   name=f"po_{qc}_{hp}_{i}")
                        for i in range(2)]
                def qk_exp(kt):
                    # both heads' scores^T into one 2-bank PSUM tile.
                    # fp8 DoubleRow: contraction pairs (K_hi, Q8) + (K_lo, Q8)
                    # -> (K_hi + K_lo)^T @ Q8 with K-side quantization error
                    # corrected; charged 0.5 cycles/row by the PE.
                    pss = ps_s.tile([128, 1024], f32, tag="pss",
                                    name=f"pss_{qc}_{hp}_{kt}")
                    for hh in range(2):
                        pb = hh * 64
                        nc.tensor.matmul(
                            pss[:, hh * 512:(hh + 1) * 512],
                            lhsT=KT_sb[pb:pb + 64, hp, :,
                                       kt * 128:(kt + 1) * 128],
                            rhs=QT_sb[pb:pb + 64, hp, :, qsl],
                            start=True, stop=True,
                            perf_mode=mybir.MatmulPerfMode.DoubleRow)
                    e = exp_pool.tile([128, 1024], bf16, tag="ex",
                                      name=f"ex_{qc}_{hp}_{kt}")
                    nc.scalar.activation(out=e, in_=pss, func=AF.Exp,
                                         scale=SCALE)
                    return e

                def av(kt, e):
                    for hh in range(2):
                        h = 2 * hp + hh
                        mm(ps_o[hh][0:65, :],
                           Vp_sb[:, kt, 65 * h:65 * h + 65],
                           e[:, hh * 512:(hh + 1) * 512],
                           kt == 0, kt == 15)

                # QK/exp run one kt ahead of AV so the pair-boundary
                # accumulator release is off the PE critical path
                e_prev = None
                for kt in range(16):
                    if fuse_v and kt % 4 == 0:
                        v_proj_chunk(kt // 4)
                    if fuse_k is not None and kt % 4 == 0:
                        k_proj_tci(fuse_k, kt // 4)
                    if fill and kt % 8 == 4:
                        fill.pop(0)()   # PE fill-in during ACT-bound stretch
                    if kt == 2:
                        # previous pair's normalization, now off the
                        # critical path (its PE replicate slots in here)
                        flush_norm()
                    e = qk_exp(kt)
                    if e_prev is not None:
                        av(kt - 1, e_prev)
                    e_prev = e
                av(15, e_prev)
                # copy accumulators (incl. denominator row) to SBUF right
                # away: frees both PSUM slots for the next pair's AVs
                oc = []
                for hh in range(2):
                    o_sb = oc_pool.tile([65, 512], f32, tag="oc",
                                        name=f"oc_{qc}_{hp}_{hh}")
                    nc.vector.tensor_copy(out=o_sb, in_=ps_o[hh][0:65, :])
                    oc.append(o_sb)
                pending_norm.append((qc, hp, oc))

            def wo_dj(qc, dj):
                qsl = slice(qc * 512, (qc + 1) * 512)
                ps_y = ps_proj.tile([128, CW], f32, tag="pp",
                                    name=f"py_{qc}_{dj}")
                for kj in range(8):
                    mm(ps_y[:, 0:512], wo_sb[:, kj, dj * 128:(dj + 1) * 128],
                       OT_tiles[qc][:, kj, :], kj == 0, kj == 7)
                yt = y_pool.tile([128, 512], f32, tag="yt",
                                 name=f"yt_{qc}_{dj}")
                nc.vector.tensor_scalar_add(
                    out=yt, in0=ps_y[:, 0:512], scalar1=bo_sb[:, dj:dj + 1])
                nc.sync.dma_start(
                    out=yT[dj * 128:(dj + 1) * 128, qsl], in_=yt)

            OT_tiles[0] = ot_pool.tile([128, 8, 512], bf16, tag="OT",
                                       name="OT_0")
            q_proj_chunk(0)        # QT for query-chunk 0
            # QT chunk 1 is produced as PE fill-in inside qc0's ACT-bound
            # head-pair loops (one dj-block per slot, hp1..hp4)
            fill_q = [lambda tci=1, dj=dj: q_proj_dj(tci, dj)
                      for dj in range(8)]
            for hp in range(8):
                fills = fill_q[2 * (hp - 1):2 * hp] if 1 <= hp <= 4 else None
                attn_hp(0, hp, fuse_v=(hp == 0), fill=fills)
            OT_tiles[1] = ot_pool.tile([128, 8, 512], bf16, tag="OT",
                                       name="OT_1")
            for hp in range(8):
                attn_hp(1, hp)
                if hp == 7:
                    # last pair's normalization before the final Wo block so
                    # its DVE chain hides under wo_dj(0,7)'s matmuls
                    flush_norm()
                wo_dj(0, hp)       # overlap qc0 output proj with qc1 attention
            for dj in range(8):
                wo_dj(1, dj)


def _prep_inputs(query, key, value, Wq, bq, Wk, bk, Wv, bv, Wo, bo):
    import ml_dtypes
    bf = ml_dtypes.bfloat16

    query = np.asarray(query, np.float32)
    key = np.asarray(key, np.float32)
    value = np.asarray(value, np.float32)
    wqT = np.ascontiguousarray(np.asarray(Wq, np.float32).T.astype(bf))
    wkT = np.ascontiguousarray(np.asarray(Wk, np.float32).T.astype(bf))
    wvT = np.ascontiguousarray(np.asarray(Wv, np.float32).T.astype(bf))
    woT = np.ascontiguousarray(np.asarray(Wo, np.float32).T.astype(bf))
    bo_eff = np.asarray(bo, np.float32) + \
        np.asarray(Wo, np.float32) @ np.asarray(bv, np.float32)
    bq_t = np.ascontiguousarray(np.asarray(bq, np.float32).reshape(8, 128).T)
    bk_t = np.ascontiguousarray(np.asarray(bk, np.float32).reshape(8, 128).T)
    bo_t = np.ascontiguousarray(bo_eff.reshape(8, 128).T)

    in_maps = []
    for c in range(N_CORES):
        b, qh = c // 2, c % 2
        in_maps.append({
            "xqT": np.ascontiguousarray(
                query[b, qh * TQ:(qh + 1) * TQ, :].T.astype(bf)),
            "xkT": np.ascontiguousarray(key[b].T.astype(bf)),
            "xvT": np.ascontiguousarray(value[b].T.astype(bf)),
            "wqT": wqT, "wkT": wkT, "wvT": wvT, "woT": woT,
            "bq_in": bq_t, "bk_in": bk_t, "bo_in": bo_t,
        })
    return in_maps


def kernel(query, key, value, Wq, bq, Wk, bk, Wv, bv, Wo, bo):
    from concourse.bass_utils import run_bass_kernel_spmd

    if "nc" not in _CACHE:
        _CACHE["nc"] = _build_program()
    nc = _CACHE["nc"]

    in_maps = _prep_inputs(query, key, value, Wq, bq, Wk, bk, Wv, bv, Wo, bo)
    res = run_bass_kernel_spmd(nc, in_maps, list(range(N_CORES)))
    out = np.empty((B, S, D), np.float32)
    for c in range(N_CORES):
        b, qh = c // 2, c % 2
        out[b, qh * TQ:(qh + 1) * TQ, :] = res.results[c]["yT"].T
    return out



# revision 32
# speedup vs baseline: 1.0777x; 1.0230x over previous
"""Multi-head attention (B=4, S=2048, D=1024, H=16) on 8 trn2 NeuronCores.

Sharding: batch x query-half. Core c handles batch c//2, query rows
(c%2)*1024 : (c%2+1)*1024. Each core projects Q for its query chunk and K/V
for the full sequence of its batch (K/V projection duplicated across the two
cores sharing a batch), runs attention for all 16 heads, and applies the
output projection. No cross-core communication.

Device-side layout notes:
 - All activations are kept transposed ([feature, token]) so every matmul
   consumes operands directly: scores are computed as S^T[k,q] = K_h^T.T @ Q_h^T,
   softmax-exp runs on ScalarE, and the AV matmul contracts over k with
   lhsT = [V_h | ones-column] (stride 65), which makes row 64 of the PSUM
   output the softmax denominator. Normalization: DVE reciprocal + a 1x64
   PE matmul to replicate it across partitions + DVE multiply. No max
   subtraction (scores are O(+-5) here, fp32 exp is safe).
 - Head pairs (2h, 2h+1) sit in partitions 0:64 / 64:128 of the same tile.
 - Both heads' scores land in one [128,1024] PSUM tile -> a single ScalarE
   exp instruction, halving ACT instruction overhead.
 - fp8e4m3 + DoubleRow (0.5 PE cycles/row) for the two biggest matmul
   groups, with one-side residual correction to stay inside the 2e-2 gate:
   * QK: contraction pairs (K_hi, Q8) + (K_lo, Q8) where K_hi = fp8(K),
     K_lo = fp8(K - K_hi): K-side quantization error cancels, leaving only
     the Q-side fp8 error (~0.8e-2 end-to-end).
   * K projection: pairs (W_hi, x8) + (16*(W-W_hi), x8/16): weight-side
     error cancels. Host pre-quantizes wk8/xk8 in the paired layout.
   V/AV and Q/Wo projections stay bf16 (measured fp8 error there busts the
   tolerance). Scores softmax drops bk entirely: softmax over keys is
   invariant to the (Q+bq)@bk term (constant per query).
 - Phase order: K proj, Q proj(qc0), then attention; V-projection and
   Q(qc1) tiles are injected one-per-kt into the first head-pairs' loops
   (fine-grained so the QK->exp pipeline never starves), qc0's output
   projection interleaves into qc1's attention, and attention QK/exp run
   one slot ahead across head-pair boundaries (pre_e0/emit_next0 handoff).
 - V bias is folded into the output-projection bias host-side
   (softmax rows sum to 1 => attn @ (V + 1 b_v^T) = attn @ V + b_v^T).
 - y is staged/stored bf16 (host upcasts); ~1e-3 extra error, halves the
   output DMA.
"""

import numpy as np

B, S, D, H = 4, 2048, 1024, 16
DK = D // H          # 64
TQ = S // 2          # per-core query tokens
TK = S               # per-core key tokens
CW = 512             # x^T streaming chunk width (tokens)
N_CORES = 8
VP_W = H * (DK + 1)  # per head: 64 V columns + 1 ones column (stride 65)
SCALE = 1.0 / np.sqrt(DK)

_CACHE = {}


def _build_program(reps=1):
    import concourse.bass as bass
    import concourse.mybir as mybir
    from concourse import bacc
    from concourse.tile import TileContext

    f32 = mybir.dt.float32
    bf16 = mybir.dt.bfloat16
    AF = mybir.ActivationFunctionType

    nc = bacc.Bacc("TRN2", target_bir_lowering=False)

    fp8d = mybir.dt.float8e4
    xqT = nc.declare_dram_parameter("xqT", [D, TQ], bf16, isOutput=False)
    # K-proj operands pre-quantized host-side for fp8 DoubleRow:
    # xk8 [p, kj, j, t] with j0 = fp8(x), j1 = fp8(x/16);
    # wk8 [p, kj, j, dout] with j0 = W_hi, j1 = 16*(W - W_hi).
    xk8 = nc.declare_dram_parameter("xk8", [128, 8, 2, TK], fp8d,
                                    isOutput=False)
    xvT = nc.declare_dram_parameter("xvT", [D, TK], bf16, isOutput=False)
    wqT = nc.declare_dram_parameter("wqT", [D, D], bf16, isOutput=False)
    wk8 = nc.declare_dram_parameter("wk8", [128, 8, 2, D], fp8d,
                                    isOutput=False)
    wvT = nc.declare_dram_parameter("wvT", [D, D], bf16, isOutput=False)
    woT = nc.declare_dram_parameter("woT", [D, D], bf16, isOutput=False)
    bq_in = nc.declare_dram_parameter("bq_in", [128, 8], f32, isOutput=False)
    bk_in = nc.declare_dram_parameter("bk_in", [128, 8], f32, isOutput=False)
    bo_in = nc.declare_dram_parameter("bo_in", [128, 8], f32, isOutput=False)
    yT = nc.declare_dram_parameter("yT", [D, TQ], bf16, isOutput=True)

    # DRAM access helpers: feature dim split as (tile j, partition p)
    xq_r = xqT[:].rearrange("(a p) t -> p a t", p=128)
    xk_r = xk8[:]
    xv_r = xvT[:].rearrange("(a p) t -> p a t", p=128)
    wq_r = wqT[:].rearrange("(a p) d -> p a d", p=128)
    wk_r = wk8[:]
    wv_r = wvT[:].rearrange("(a p) d -> p a d", p=128)
    wo_r = woT[:].rearrange("(a p) d -> p a d", p=128)

    with TileContext(nc) as tc:
        for _rep in range(reps):
            _emit_body(nc, tc, bass, f32, bf16, AF,
                       xq_r, xk_r, xv_r, wq_r, wk_r, wv_r, wo_r,
                       bq_in, bk_in, bo_in, yT)
    nc.compile()
    return nc


def _emit_body(nc, tc, bass, f32, bf16, AF,
               xq_r, xk_r, xv_r, wq_r, wk_r, wv_r, wo_r,
               bq_in, bk_in, bo_in, yT):
    import concourse.mybir as mybir
    def mm(out, lhsT, rhs, start, stop):
        nc.tensor.matmul(out, lhsT=lhsT, rhs=rhs, start=start, stop=stop)

    if True:
        with (
            tc.tile_pool(name="const", bufs=1) as const_pool,
            tc.tile_pool(name="kt_res", bufs=1) as kt_pool,
            tc.tile_pool(name="qt_res", bufs=1) as qt_pool,
            tc.tile_pool(name="vp_res", bufs=1) as vp_pool,
            tc.tile_pool(name="ot_res", bufs=2) as ot_pool,
            tc.tile_pool(name="w_res", bufs=2) as w_pool,
            tc.tile_pool(name="x_str", bufs=2) as x_pool,
            tc.tile_pool(name="exp_p", bufs=3) as exp_pool,
            tc.tile_pool(name="rec_p", bufs=1) as rec_pool,
            tc.tile_pool(name="recb_p", bufs=2) as recb_pool,
            tc.tile_pool(name="oc_p", bufs=4) as oc_pool,
            tc.tile_pool(name="y_p", bufs=2) as y_pool,
            tc.tile_pool(name="ps_proj", bufs=2, space="PSUM") as ps_proj,
            tc.tile_pool(name="ps_s", bufs=2, space="PSUM") as ps_s,
            tc.tile_pool(name="ps_av", bufs=2, space="PSUM") as ps_av,
        ):
            bq_sb = const_pool.tile([128, 8], f32, tag="bq")
            bk_sb = const_pool.tile([128, 8], f32, tag="bk")
            bo_sb = const_pool.tile([128, 8], f32, tag="bo")
            # separate queue: keep these tiny loads off the head of the
            # sync queue that feeds the first matmuls
            nc.gpsimd.dma_start(out=bq_sb, in_=bq_in[:])
            nc.gpsimd.dma_start(out=bk_sb, in_=bk_in[:])
            nc.gpsimd.dma_start(out=bo_sb, in_=bo_in[:])

            fp8 = mybir.dt.float8e4
            # [p, hp, j, t]: j=0 -> K_hi / Q8, j=1 -> K_lo residual / Q8 dup
            KT_sb = kt_pool.tile([128, 8, 2, TK], fp8, tag="KT")
            QT_sb = qt_pool.tile([128, 8, 2, TQ], fp8, tag="QT")
            Vp_sb = vp_pool.tile([128, 16, VP_W], bf16, tag="Vp")  # [p, i, c]
            # view: [p, ktile, head, col(65)]
            Vp4 = Vp_sb.rearrange("p i (hh c) -> p i hh c", c=DK + 1)
            nc.vector.memset(Vp4[:, :, :, DK], 1.0)
            ones_sb = const_pool.tile([1, 64], bf16, tag="ones")
            nc.vector.memset(ones_sb, 1.0)

            # ---- K projection: K^T[dout, t], streamed x chunks ----
            # split loads so the first dj-column's matmuls start early
            # K projection in fp8 DoubleRow: pairs (W_hi, x8) + (W_lo*16,
            # x8/16) -> W-side quantization corrected, x-side ~fp8. 0.5
            # cycles/row => K proj at half bf16 PE cost.
            wk_sb = w_pool.tile([128, 8, 2, D], fp8, tag="wk8", name="wk_sb",
                                bufs=1)
            nc.sync.dma_start(out=wk_sb[:, :, :, 0:128],
                              in_=wk_r[:, :, :, 0:128])
            for tci in range(TK // CW):
                xc = x_pool.tile([128, 8, 2, CW], fp8, tag="xchunk",
                                 name=f"xk_{tci}")
                if tci == 0:
                    for kq in range(4):
                        nc.sync.dma_start(
                            out=xc[:, 2 * kq:2 * kq + 2, :, :],
                            in_=xk_r[:, 2 * kq:2 * kq + 2, :, 0:CW])
                    # remaining K-weight columns, one dj-slice each, queued
                    # behind the first activation chunk
                    for djw in range(1, 8):
                        nc.sync.dma_start(
                            out=wk_sb[:, :, :, djw * 128:(djw + 1) * 128],
                            in_=wk_r[:, :, :, djw * 128:(djw + 1) * 128])
                else:
                    nc.sync.dma_start(
                        out=xc, in_=xk_r[:, :, :, tci * CW:(tci + 1) * CW])
                for dj in range(8):
                    ps = ps_proj.tile([128, CW], f32, tag="pp",
                                      name=f"pk_{tci}_{dj}")
                    for kj in range(8):
                        nc.tensor.matmul(
                            ps, lhsT=wk_sb[:, kj, :, dj * 128:(dj + 1) * 128],
                            rhs=xc[:, kj, :, :], start=kj == 0, stop=kj == 7,
                            perf_mode=mybir.MatmulPerfMode.DoubleRow)
                    # bk is dropped: softmax over keys is invariant to the
                    # (Q+bq)@bk term (constant per query). K_hi = fp8(K),
                    # K_lo = fp8(K - K_hi) -> DoubleRow pair corrects the
                    # K-side quantization error.
                    ksl = slice(tci * CW, (tci + 1) * CW)
                    nc.vector.tensor_copy(out=KT_sb[:, dj, 0, ksl], in_=ps)
                    with nc.allow_low_precision(
                            reason="fp8 residual capture for DoubleRow QK"):
                        nc.vector.tensor_sub(out=KT_sb[:, dj, 1, ksl],
                                             in0=ps, in1=KT_sb[:, dj, 0, ksl])

            # ---- Q projection ----
            wq_sb = w_pool.tile([128, 8, D], bf16, tag="wbig", name="wq_sb")
            nc.sync.dma_start(out=wq_sb, in_=wq_r)

            xq_tiles = {}

            def q_proj_dj(tci, dj):
                if dj == 0:
                    xq_tiles[tci] = x_pool.tile([128, 8, CW], bf16,
                                                tag="xchunk",
                                                name=f"xq_{tci}")
                    nc.sync.dma_start(
                        out=xq_tiles[tci],
                        in_=xq_r[:, :, tci * CW:(tci + 1) * CW])
                xc = xq_tiles[tci]
                ps = ps_proj.tile([128, CW], f32, tag="pp",
                                  name=f"pq_{tci}_{dj}")
                for kj in range(8):
                    mm(ps, wq_sb[:, kj, dj * 128:(dj + 1) * 128],
                       xc[:, kj, :], kj == 0, kj == 7)
                qsl = slice(tci * CW, (tci + 1) * CW)
                nc.vector.tensor_scalar_add(
                    out=QT_sb[:, dj, 0, qsl], in0=ps,
                    scalar1=bq_sb[:, dj:dj + 1])
                # duplicate Q8 into the second DoubleRow slot (pairs with
                # K_lo); cheap 16-bit-view copy
                nc.vector.tensor_copy(
                    out=QT_sb[:, dj, 1, qsl].bitcast(mybir.dt.uint16),
                    in_=QT_sb[:, dj, 0, qsl].bitcast(mybir.dt.uint16))

            def q_proj_chunk(tci):
                for dj in range(8):
                    q_proj_dj(tci, dj)

            # V and Wo weights resident; V projection is fused into the first
            # head-pair's attention loop below so ScalarE exp overlaps it
            wv_sb = w_pool.tile([128, 8, D], bf16, tag="wbig", name="wv_sb")
            nc.sync.dma_start(out=wv_sb, in_=wv_r)
            wo_sb = None  # loaded just before qc1 (reuses wq's buffer)

            xv_tiles = {}

            def xv_dma(tci):
                xc = x_pool.tile([128, 8, CW], bf16, tag="xvchunk",
                                 name=f"xv_{tci}", bufs=4)
                nc.sync.dma_start(out=xc,
                                  in_=xv_r[:, :, tci * CW:(tci + 1) * CW])
                xv_tiles[tci] = xc

            def v_proj_tile(ti, dc):
                xc = xv_tiles[ti // 4]
                ts2 = ti % 4
                ps = ps_proj.tile([128, CW], f32, tag="pp",
                                  name=f"pv_{ti}_{dc}")
                for kj in range(8):
                    mm(ps[:, 0:512], xc[:, kj, ts2 * 128:(ts2 + 1) * 128],
                       wv_sb[:, kj, dc * 512:(dc + 1) * 512],
                       kj == 0, kj == 7)
                nc.vector.tensor_copy(
                    out=Vp4[:, ti, dc * 8:(dc + 1) * 8, 0:DK],
                    in_=ps[:, 0:512].rearrange("p (hh c) -> p hh c", c=DK))

            OT_tiles = {}

            # deferred normalization: (qc, hp, sbuf copy of [65,512] accum)
            pending_norm = []

            def flush_norm():
                while pending_norm:
                    qc, hp, oc = pending_norm.pop(0)
                    for hh in range(2):
                        # row 64 of oc = softmax denominator
                        rec = rec_pool.tile([1, 512], bf16, tag="rec",
                                            name=f"rec_{qc}_{hp}_{hh}")
                        with nc.allow_low_precision(
                                reason="softmax denom reciprocal, bf16 "
                                       "matches pipeline precision"):
                            nc.vector.reciprocal(out=rec,
                                                 in_=oc[hh][64:65, :])
                        # replicate reciprocal across 64 partitions via PE
                        ps_rep = ps_proj.tile([128, CW], f32, tag="pp",
                                              name=f"pr_{qc}_{hp}_{hh}")
                        mm(ps_rep[0:64, 0:512], ones_sb, rec, True, True)
                        recb = recb_pool.tile([64, 512], bf16, tag="recb",
                                              name=f"recb_{qc}_{hp}_{hh}")
                        nc.vector.tensor_copy(out=recb,
                                              in_=ps_rep[0:64, 0:512])
                        nc.vector.tensor_mul(
                            out=OT_tiles[qc][hh * 64:(hh + 1) * 64, hp, :],
                            in0=oc[hh][0:64, :], in1=recb)

            def qk_exp(qc, hp, kt):
                # both heads' scores^T into one 2-bank PSUM tile.
                # fp8 DoubleRow: contraction pairs (K_hi, Q8) + (K_lo, Q8)
                # -> (K_hi + K_lo)^T @ Q8 with K-side quantization error
                # corrected; charged 0.5 cycles/row by the PE.
                qsl = slice(qc * 512, (qc + 1) * 512)
                pss = ps_s.tile([128, 1024], f32, tag="pss",
                                name=f"pss_{qc}_{hp}_{kt}")
                for hh in range(2):
                    pb = hh * 64
                    nc.tensor.matmul(
                        pss[:, hh * 512:(hh + 1) * 512],
                        lhsT=KT_sb[pb:pb + 64, hp, :,
                                   kt * 128:(kt + 1) * 128],
                        rhs=QT_sb[pb:pb + 64, hp, :, qsl],
                        start=True, stop=True,
                        perf_mode=mybir.MatmulPerfMode.DoubleRow)
                e = exp_pool.tile([128, 1024], bf16, tag="ex",
                                  name=f"ex_{qc}_{hp}_{kt}")
                nc.scalar.activation(out=e, in_=pss, func=AF.Exp,
                                     scale=SCALE)
                return e

            def attn_hp(qc, hp, inject=None, pre_e0=None, emit_next0=None):
                ps_o = [ps_av.tile([128, 512], f32, tag="po",
                                   name=f"po_{qc}_{hp}_{i}")
                        for i in range(2)]

                def av(kt, e):
                    for hh in range(2):
                        h = 2 * hp + hh
                        mm(ps_o[hh][0:65, :],
                           Vp_sb[:, kt, 65 * h:65 * h + 65],
                           e[:, hh * 512:(hh + 1) * 512],
                           kt == 0, kt == 15)

                # QK/exp run one kt ahead of AV (and one slot across the
                # head-pair boundary via pre_e0/emit_next0) so ScalarE never
                # waits for the boundary accumulator handoff.
                e_prev = pre_e0
                kt0 = 0
                if pre_e0 is not None:
                    kt0 = 1
                    if inject is not None and inject.get(0) is not None:
                        inject[0]()
                for kt in range(kt0, 16):
                    if inject is not None and inject.get(kt) is not None:
                        inject[kt]()   # fine-grained PE fill-in (~1.7us max)
                    if kt == 2:
                        # previous pair's normalization, now off the
                        # critical path
                        flush_norm()
                    e = qk_exp(qc, hp, kt)
                    if e_prev is not None:
                        av(kt - 1, e_prev)
                    e_prev = e
                next_e = emit_next0() if emit_next0 is not None else None
                av(15, e_prev)
                # copy accumulators (incl. denominator row) to SBUF right
                # away: frees both PSUM slots for the next pair's AVs
                oc = []
                for hh in range(2):
                    o_sb = oc_pool.tile([65, 512], bf16, tag="oc",
                                        name=f"oc_{qc}_{hp}_{hh}")
                    nc.vector.tensor_copy(out=o_sb, in_=ps_o[hh][0:65, :])
                    oc.append(o_sb)
                pending_norm.append((qc, hp, oc))
                return next_e

            def wo_dj(qc, dj):
                qsl = slice(qc * 512, (qc + 1) * 512)
                ps_y = ps_proj.tile([128, CW], f32, tag="pp",
                                    name=f"py_{qc}_{dj}")
                for kj in range(8):
                    mm(ps_y[:, 0:512], wo_sb[:, kj, dj * 128:(dj + 1) * 128],
                       OT_tiles[qc][:, kj, :], kj == 0, kj == 7)
                yt = y_pool.tile([128, 512], bf16, tag="yt",
                                 name=f"yt_{qc}_{dj}")
                if qc == 1:
                    # tail: ACT is idle after attention ends; bias-add there
                    nc.scalar.activation(
                        out=yt, in_=ps_y[:, 0:512],
                        func=AF.Identity, bias=bo_sb[:, dj:dj + 1])
                else:
                    nc.vector.tensor_scalar_add(
                        out=yt, in0=ps_y[:, 0:512], scalar1=bo_sb[:, dj:dj + 1])
                nc.sync.dma_start(
                    out=yT[dj * 128:(dj + 1) * 128, qsl], in_=yt)

            OT_tiles[0] = ot_pool.tile([128, 8, 512], bf16, tag="OT",
                                       name="OT_0")
            q_proj_chunk(0)        # QT for query-chunk 0
            for tci in range(4):   # prefetch all xv chunks early
                xv_dma(tci)
            # Fine-grained PE fill-in inside qc0's ACT-bound head-pair loops:
            # one V-proj tile (~1.7us) or Q-proj dj block per kt slot, so the
            # QK->exp pipeline never starves for more than the 2-tile exp
            # buffer.
            inj = {
                0: {kt: (lambda ti=kt: v_proj_tile(ti, 0)) for kt in range(16)},
                1: {kt: (lambda ti=kt: v_proj_tile(ti, 1)) for kt in range(16)},
                2: {kt: (lambda dj=kt // 2: q_proj_dj(1, dj))
                    for kt in range(0, 16, 2)},
            }
            pre_e = None
            for hp in range(8):
                nxt = ((lambda h=hp + 1: qk_exp(0, h, 0)) if hp < 7
                       else (lambda: qk_exp(1, 0, 0)))
                pre_e = attn_hp(0, hp, inject=inj.get(hp), pre_e0=pre_e,
                                emit_next0=nxt)
            wo_sb = w_pool.tile([128, 8, D], bf16, tag="wbig", name="wo_sb")
            nc.sync.dma_start(out=wo_sb, in_=wo_r)
            OT_tiles[1] = ot_pool.tile([128, 8, 512], bf16, tag="OT",
                                       name="OT_1")
            for hp in range(8):
                nxt = (lambda h=hp + 1: qk_exp(1, h, 0)) if hp < 7 else None
                pre_e = attn_hp(1, hp, pre_e0=pre_e, emit_next0=nxt)
                if hp == 7:
                    # last pair's normalization before the final Wo block so
                    # its DVE chain hides under wo_dj(0,7)'s matmuls
                    flush_norm()
                wo_dj(0, hp)       # overlap qc0 output proj with qc1 attention
            for dj in range(8):
                wo_dj(1, dj)


def _xk8(kb):
    """[p, kj, j, t] fp8 pair layout: j0 = fp8(x), j1 = fp8(x/16)."""
    import ml_dtypes
    f8 = ml_dtypes.float8_e4m3
    xk = np.asarray(kb, np.float32).T            # [din, t]
    out = np.empty((128, 8, 2, xk.shape[1]), dtype=f8)
    out[:, :, 0, :] = xk.astype(f8).reshape(8, 128, -1).transpose(1, 0, 2)
    out[:, :, 1, :] = (xk / 16).astype(f8).reshape(8, 128, -1).transpose(1, 0, 2)
    return out


def _prep_inputs(query, key, value, Wq, bq, Wk, bk, Wv, bv, Wo, bo):
    import ml_dtypes
    bf = ml_dtypes.bfloat16

    query = np.asarray(query, np.float32)
    key = np.asarray(key, np.float32)
    value = np.asarray(value, np.float32)
    f8 = ml_dtypes.float8_e4m3
    wqT = np.ascontiguousarray(np.asarray(Wq, np.float32).T.astype(bf))
    WkT = np.asarray(Wk, np.float32).T           # [din, dout]
    wk_hi = WkT.astype(f8)
    wk_lo = ((WkT - wk_hi.astype(np.float32)) * 16).astype(f8)
    wk8 = np.empty((128, 8, 2, 1024), dtype=f8)  # [p, kj, j, dout]
    wk8[:, :, 0, :] = wk_hi.reshape(8, 128, 1024).transpose(1, 0, 2)
    wk8[:, :, 1, :] = wk_lo.reshape(8, 128, 1024).transpose(1, 0, 2)
    wvT = np.ascontiguousarray(np.asarray(Wv, np.float32).T.astype(bf))
    woT = np.ascontiguousarray(np.asarray(Wo, np.float32).T.astype(bf))
    bo_eff = np.asarray(bo, np.float32) + \
        np.asarray(Wo, np.float32) @ np.asarray(bv, np.float32)
    bq_t = np.ascontiguousarray(np.asarray(bq, np.float32).reshape(8, 128).T)
    bk_t = np.ascontiguousarray(np.asarray(bk, np.float32).reshape(8, 128).T)
    bo_t = np.ascontiguousarray(bo_eff.reshape(8, 128).T)

    in_maps = []
    for c in range(N_CORES):
        b, qh = c // 2, c % 2
        in_maps.append({
            "xqT": np.ascontiguousarray(
                query[b, qh * TQ:(qh + 1) * TQ, :].T.astype(bf)),
            "xk8": _xk8(key[b]),
            "xvT": np.ascontiguousarray(value[b].T.astype(bf)),
            "wqT": wqT, "wk8": wk8, "wvT": wvT, "woT": woT,
            "bq_in": bq_t, "bk_in": bk_t, "bo_in": bo_t,
        })
    return in_maps


def kernel(query, key, value, Wq, bq, Wk, bk, Wv, bv, Wo, bo):
    from concourse.bass_utils import run_bass_kernel_spmd

    if "nc" not in _CACHE:
        _CACHE["nc"] = _build_program()
    nc = _CACHE["nc"]

    in_maps = _prep_inputs(query, key, value, Wq, bq, Wk, bk, Wv, bv, Wo, bo)
    res = run_bass_kernel_spmd(nc, in_maps, list(range(N_CORES)))
    out = np.empty((B, S, D), np.float32)
    for c in range(N_CORES):
        b, qh = c // 2, c % 2
        out[b, qh * TQ:(qh + 1) * TQ, :] = \
            res.results[c]["yT"].T.astype(np.float32)
    return out



# revision 38
# speedup vs baseline: 1.1146x; 1.0342x over previous
"""Multi-head attention (B=4, S=2048, D=1024, H=16) on 8 trn2 NeuronCores.

Sharding: batch x query-half. Core c handles batch c//2, query rows
(c%2)*1024 : (c%2+1)*1024. Each core projects Q for its query chunk and K/V
for the full sequence of its batch (K/V projection duplicated across the two
cores sharing a batch), runs attention for all 16 heads, and applies the
output projection. No cross-core communication.

Device-side layout notes:
 - All activations are kept transposed ([feature, token]) so every matmul
   consumes operands directly: scores are computed as S^T[k,q] = K_h^T.T @ Q_h^T,
   softmax-exp runs on ScalarE, and the AV matmul contracts over k with
   lhsT = [V_h | ones-column] (stride 65), which makes row 64 of the PSUM
   output the softmax denominator. Normalization: DVE reciprocal + a 1x64
   PE matmul to replicate it across partitions + DVE multiply. No max
   subtraction (scores are O(+-5) here, fp32 exp is safe).
 - Head pairs (2h, 2h+1) sit in partitions 0:64 / 64:128 of the same tile.
 - Both heads' scores land in one [128,1024] PSUM tile -> a single ScalarE
   exp instruction, halving ACT instruction overhead.
 - fp8e4m3 + DoubleRow (0.5 PE cycles/row) for the two biggest matmul
   groups, with one-side residual correction to stay inside the 2e-2 gate:
   * QK: contraction pairs (K_hi, Q8) + (K_lo, Q8) where K_hi = fp8(K),
     K_lo = fp8(K - K_hi): K-side quantization error cancels, leaving only
     the Q-side fp8 error (~0.8e-2 end-to-end).
   * K projection: pairs (W_hi, x8) + (16*(W-W_hi), x8/16): weight-side
     error cancels. Host pre-quantizes wk8/xk8 in the paired layout.
   V/AV and Q/Wo projections stay bf16 (measured fp8 error there busts the
   tolerance). Scores softmax drops bk entirely: softmax over keys is
   invariant to the (Q+bq)@bk term (constant per query).
 - Phase order: K proj, Q proj(qc0), then attention; V-projection and
   Q(qc1) tiles are injected one-per-kt into the first head-pairs' loops
   (fine-grained so the QK->exp pipeline never starves), qc0's output
   projection interleaves into qc1's attention, and attention QK/exp run
   one slot ahead across head-pair boundaries (pre_e0/emit_next0 handoff).
 - V bias is folded into the output-projection bias host-side
   (softmax rows sum to 1 => attn @ (V + 1 b_v^T) = attn @ V + b_v^T).
 - y is staged/stored bf16 (host upcasts); ~1e-3 extra error, halves the
   output DMA.
"""

import numpy as np

B, S, D, H = 4, 2048, 1024, 16
DK = D // H          # 64
TQ = S // 2          # per-core query tokens
TK = S               # per-core key tokens
CW = 512             # x^T streaming chunk width (tokens)
N_CORES = 8
VP_W = H * (DK + 1)  # per head: 64 V columns + 1 ones column (stride 65)
SCALE = 1.0 / np.sqrt(DK)

_CACHE = {}


def _build_program(reps=1):
    import concourse.bass as bass
    import concourse.mybir as mybir
    from concourse import bacc
    from concourse.tile import TileContext

    f32 = mybir.dt.float32
    bf16 = mybir.dt.bfloat16
    AF = mybir.ActivationFunctionType

    nc = bacc.Bacc("TRN2", target_bir_lowering=False)

    fp8d = mybir.dt.float8e4
    xqT = nc.declare_dram_parameter("xqT", [D, TQ], bf16, isOutput=False)
    # K-proj operands pre-quantized host-side for fp8 DoubleRow:
    # xk8 [p, kj, j, t] with j0 = fp8(x), j1 = fp8(x/16);
    # wk8 [p, kj, j, dout] with j0 = W_hi, j1 = 16*(W - W_hi).
    xk8 = nc.declare_dram_parameter("xk8", [128, 8, 2, TK], fp8d,
                                    isOutput=False)
    xvT = nc.declare_dram_parameter("xvT", [D, TK], bf16, isOutput=False)
    wqT = nc.declare_dram_parameter("wqT", [D, D], bf16, isOutput=False)
    wk8 = nc.declare_dram_parameter("wk8", [128, 8, 2, D], fp8d,
                                    isOutput=False)
    wvT = nc.declare_dram_parameter("wvT", [D, D], bf16, isOutput=False)
    woT = nc.declare_dram_parameter("woT", [D, D], bf16, isOutput=False)
    bq_in = nc.declare_dram_parameter("bq_in", [128, 8], f32, isOutput=False)
    bk_in = nc.declare_dram_parameter("bk_in", [128, 8], f32, isOutput=False)
    bo_in = nc.declare_dram_parameter("bo_in", [128, 8], f32, isOutput=False)
    yT = nc.declare_dram_parameter("yT", [D, TQ], bf16, isOutput=True)

    # DRAM access helpers: feature dim split as (tile j, partition p)
    xq_r = xqT[:].rearrange("(a p) t -> p a t", p=128)
    xk_r = xk8[:]
    xv_r = xvT[:].rearrange("(a p) t -> p a t", p=128)
    wq_r = wqT[:].rearrange("(a p) d -> p a d", p=128)
    wk_r = wk8[:]
    wv_r = wvT[:].rearrange("(a p) d -> p a d", p=128)
    wo_r = woT[:].rearrange("(a p) d -> p a d", p=128)

    with TileContext(nc) as tc:
        for _rep in range(reps):
            _emit_body(nc, tc, bass, f32, bf16, AF,
                       xq_r, xk_r, xv_r, wq_r, wk_r, wv_r, wo_r,
                       bq_in, bk_in, bo_in, yT)
    nc.compile()
    return nc


def _emit_body(nc, tc, bass, f32, bf16, AF,
               xq_r, xk_r, xv_r, wq_r, wk_r, wv_r, wo_r,
               bq_in, bk_in, bo_in, yT):
    import concourse.mybir as mybir
    def mm(out, lhsT, rhs, start, stop):
        nc.tensor.matmul(out, lhsT=lhsT, rhs=rhs, start=start, stop=stop)

    if True:
        with (
            tc.tile_pool(name="const", bufs=1) as const_pool,
            tc.tile_pool(name="kt_res", bufs=1) as kt_pool,
            tc.tile_pool(name="qt_res", bufs=1) as qt_pool,
            tc.tile_pool(name="vp_res", bufs=1) as vp_pool,
            tc.tile_pool(name="ot_res", bufs=2) as ot_pool,
            tc.tile_pool(name="w_res", bufs=2) as w_pool,
            tc.tile_pool(name="x_str", bufs=2) as x_pool,
            tc.tile_pool(name="exp_p", bufs=3) as exp_pool,
            tc.tile_pool(name="rec_p", bufs=1) as rec_pool,
            tc.tile_pool(name="recb_p", bufs=2) as recb_pool,
            tc.tile_pool(name="oc_p", bufs=4) as oc_pool,
            tc.tile_pool(name="y_p", bufs=2) as y_pool,
            tc.tile_pool(name="ps_proj", bufs=2, space="PSUM") as ps_proj,
            tc.tile_pool(name="ps_s", bufs=2, space="PSUM") as ps_s,
            tc.tile_pool(name="ps_av", bufs=2, space="PSUM") as ps_av,
        ):
            bq_sb = const_pool.tile([128, 8], f32, tag="bq")
            bk_sb = const_pool.tile([128, 8], f32, tag="bk")
            bo_sb = const_pool.tile([128, 8], f32, tag="bo")
            # separate queue: keep these tiny loads off the head of the
            # sync queue that feeds the first matmuls
            nc.gpsimd.dma_start(out=bq_sb, in_=bq_in[:])
            nc.gpsimd.dma_start(out=bk_sb, in_=bk_in[:])
            nc.gpsimd.dma_start(out=bo_sb, in_=bo_in[:])

            fp8 = mybir.dt.float8e4
            # [p, hp, j, t]: j=0 -> K_hi / Q8, j=1 -> K_lo residual / Q8 dup
            KT_sb = kt_pool.tile([128, 8, 2, TK], fp8, tag="KT")
            QT_sb = qt_pool.tile([128, 8, 2, TQ], fp8, tag="QT")
            Vp_sb = vp_pool.tile([128, 16, VP_W], bf16, tag="Vp")  # [p, i, c]
            # view: [p, ktile, head, col(65)]
            Vp4 = Vp_sb.rearrange("p i (hh c) -> p i hh c", c=DK + 1)
            nc.vector.memset(Vp4[:, :, :, DK], 1.0)
            ones_sb = const_pool.tile([1, 64], bf16, tag="ones")
            nc.vector.memset(ones_sb, 1.0)

            # ---- K projection: K^T[dout, t], streamed x chunks ----
            # split loads so the first dj-column's matmuls start early
            # K projection in fp8 DoubleRow: pairs (W_hi, x8) + (W_lo*16,
            # x8/16) -> W-side quantization corrected, x-side ~fp8. 0.5
            # cycles/row => K proj at half bf16 PE cost.
            wq_sb = w_pool.tile([128, 8, D], bf16, tag="wbig", name="wq_sb")
            xq_tiles = {}
            wk_sb = w_pool.tile([128, 8, 2, D], fp8, tag="wk8", name="wk_sb",
                                bufs=1)
            def q_proj_dj(tci, dj):
                if dj == 0:
                    xq_tiles[tci] = x_pool.tile([128, 8, CW], bf16,
                                                tag="xchunk",
                                                name=f"xq_{tci}")
                    nc.sync.dma_start(
                        out=xq_tiles[tci],
                        in_=xq_r[:, :, tci * CW:(tci + 1) * CW])
                xc = xq_tiles[tci]
                ps = ps_proj.tile([128, CW], f32, tag="pp",
                                  name=f"pq_{tci}_{dj}")
                for kj in range(8):
                    mm(ps, wq_sb[:, kj, dj * 128:(dj + 1) * 128],
                       xc[:, kj, :], kj == 0, kj == 7)
                qsl = slice(tci * CW, (tci + 1) * CW)
                nc.vector.tensor_scalar_add(
                    out=QT_sb[:, dj, 0, qsl], in0=ps,
                    scalar1=bq_sb[:, dj:dj + 1])
                # duplicate Q8 into the second DoubleRow slot (pairs with
                # K_lo); cheap 16-bit-view copy
                nc.vector.tensor_copy(
                    out=QT_sb[:, dj, 1, qsl].bitcast(mybir.dt.uint16),
                    in_=QT_sb[:, dj, 0, qsl].bitcast(mybir.dt.uint16))

            nc.sync.dma_start(out=wk_sb[:, :, :, 0:128],
                              in_=wk_r[:, :, :, 0:128])
            for tci in range(TK // CW):
                xc = x_pool.tile([128, 8, 2, CW], fp8, tag="xchunk",
                                 name=f"xk_{tci}")
                if tci == 0:
                    for kq in range(4):
                        nc.sync.dma_start(
                            out=xc[:, 2 * kq:2 * kq + 2, :, :],
                            in_=xk_r[:, 2 * kq:2 * kq + 2, :, 0:CW])
                    # remaining K-weight columns, one dj-slice each, queued
                    # behind the first activation chunk
                    for djw in range(1, 8):
                        nc.sync.dma_start(
                            out=wk_sb[:, :, :, djw * 128:(djw + 1) * 128],
                            in_=wk_r[:, :, :, djw * 128:(djw + 1) * 128])
                else:
                    nc.sync.dma_start(
                        out=xc, in_=xk_r[:, :, :, tci * CW:(tci + 1) * CW])
                for dj in range(8):
                    ps = ps_proj.tile([128, CW], f32, tag="pp",
                                      name=f"pk_{tci}_{dj}")
                    for kj in range(8):
                        nc.tensor.matmul(
                            ps, lhsT=wk_sb[:, kj, :, dj * 128:(dj + 1) * 128],
                            rhs=xc[:, kj, :, :], start=kj == 0, stop=kj == 7,
                            perf_mode=mybir.MatmulPerfMode.DoubleRow)
                    # bk is dropped: softmax over keys is invariant to the
                    # (Q+bq)@bk term (constant per query). K_hi = fp8(K),
                    # K_lo = fp8(K - K_hi) -> DoubleRow pair corrects the
                    # K-side quantization error.
                    ksl = slice(tci * CW, (tci + 1) * CW)
                    # K_hi evac on ScalarE: it is idle during the projection
                    # phase, and this halves the DVE chain that was binding it
                    nc.scalar.activation(out=KT_sb[:, dj, 0, ksl], in_=ps,
                                         func=AF.Copy)
                    with nc.allow_low_precision(
                            reason="fp8 residual capture for DoubleRow QK"):
                        nc.vector.tensor_sub(out=KT_sb[:, dj, 1, ksl],
                                             in0=ps, in1=KT_sb[:, dj, 0, ksl])


            # V and Wo weights resident; V projection is fused into the first
            # head-pair's attention loop below so ScalarE exp overlaps it
            wv_sb = w_pool.tile([128, 8, D], bf16, tag="wbig", name="wv_sb")
            nc.sync.dma_start(out=wv_sb, in_=wv_r)
            wo_sb = None  # loaded just before qc1 (reuses wq's buffer)

            xv_tiles = {}

            def xv_dma(tci):
                xc = x_pool.tile([128, 8, CW], bf16, tag="xvchunk",
                                 name=f"xv_{tci}", bufs=4)
                nc.sync.dma_start(out=xc,
                                  in_=xv_r[:, :, tci * CW:(tci + 1) * CW])
                xv_tiles[tci] = xc

            def v_proj_tile(ti, dc):
                xc = xv_tiles[ti // 4]
                ts2 = ti % 4
                ps = ps_proj.tile([128, CW], f32, tag="pp",
                                  name=f"pv_{ti}_{dc}")
                for kj in range(8):
                    mm(ps[:, 0:512], xc[:, kj, ts2 * 128:(ts2 + 1) * 128],
                       wv_sb[:, kj, dc * 512:(dc + 1) * 512],
                       kj == 0, kj == 7)
                nc.vector.tensor_copy(
                    out=Vp4[:, ti, dc * 8:(dc + 1) * 8, 0:DK],
                    in_=ps[:, 0:512].rearrange("p (hh c) -> p hh c", c=DK))

            OT_tiles = {}

            # deferred normalization: (qc, hp, sbuf copy of [65,512] accum)
            pending_norm = []

            def flush_norm():
                while pending_norm:
                    qc, hp, oc = pending_norm.pop(0)
                    for hh in range(2):
                        # row 64 of oc = softmax denominator
                        rec = rec_pool.tile([1, 512], bf16, tag="rec",
                                            name=f"rec_{qc}_{hp}_{hh}")
                        with nc.allow_low_precision(
                                reason="softmax denom reciprocal, bf16 "
                                       "matches pipeline precision"):
                            nc.vector.reciprocal(out=rec,
                                                 in_=oc[hh][64:65, :])
                        # replicate reciprocal across 64 partitions via PE
                        ps_rep = ps_proj.tile([128, CW], f32, tag="pp",
                                              name=f"pr_{qc}_{hp}_{hh}")
                        mm(ps_rep[0:64, 0:512], ones_sb, rec, True, True)
                        recb = recb_pool.tile([64, 512], bf16, tag="recb",
                                              name=f"recb_{qc}_{hp}_{hh}")
                        nc.vector.tensor_copy(out=recb,
                                              in_=ps_rep[0:64, 0:512])
                        nc.vector.tensor_mul(
                            out=OT_tiles[qc][hh * 64:(hh + 1) * 64, hp, :],
                            in0=oc[hh][0:64, :], in1=recb)

            def qk_exp(qc, hp, kt):
                # both heads' scores^T into one 2-bank PSUM tile.
                # fp8 DoubleRow: contraction pairs (K_hi, Q8) + (K_lo, Q8)
                # -> (K_hi + K_lo)^T @ Q8 with K-side quantization error
                # corrected; charged 0.5 cycles/row by the PE.
                qsl = slice(qc * 512, (qc + 1) * 512)
                pss = ps_s.tile([128, 1024], f32, tag="pss",
                                name=f"pss_{qc}_{hp}_{kt}")
                for hh in range(2):
                    pb = hh * 64
                    nc.tensor.matmul(
                        pss[:, hh * 512:(hh + 1) * 512],
                        lhsT=KT_sb[pb:pb + 64, hp, :,
                                   kt * 128:(kt + 1) * 128],
                        rhs=QT_sb[pb:pb + 64, hp, :, qsl],
                        start=True, stop=True,
                        perf_mode=mybir.MatmulPerfMode.DoubleRow)
                e = exp_pool.tile([128, 1024], bf16, tag="ex",
                                  name=f"ex_{qc}_{hp}_{kt}")
                nc.scalar.activation(out=e, in_=pss, func=AF.Exp,
                                     scale=SCALE)
                return e

            def attn_hp(qc, hp, inject=None, pre_e0=None, emit_next0=None):
                ps_o = [ps_av.tile([128, 512], f32, tag="po",
                                   name=f"po_{qc}_{hp}_{i}")
                        for i in range(2)]

                def av(kt, e):
                    for hh in range(2):
                        h = 2 * hp + hh
                        mm(ps_o[hh][0:65, :],
                           Vp_sb[:, kt, 65 * h:65 * h + 65],
                           e[:, hh * 512:(hh + 1) * 512],
                           kt == 0, kt == 15)

                # QK/exp run one kt ahead of AV (and one slot across the
                # head-pair boundary via pre_e0/emit_next0) so ScalarE never
                # waits for the boundary accumulator handoff.
                e_prev = pre_e0
                kt0 = 0
                if pre_e0 is not None:
                    kt0 = 1
                    if inject is not None and inject.get(0) is not None:
                        inject[0]()
                for kt in range(kt0, 16):
                    if inject is not None and inject.get(kt) is not None:
                        inject[kt]()   # fine-grained PE fill-in (~1.7us max)
                    if kt == 2:
                        # previous pair's normalization, now off the
                        # critical path
                        flush_norm()
                    e = qk_exp(qc, hp, kt)
                    if e_prev is not None:
                        av(kt - 1, e_prev)
                    e_prev = e
                next_e = emit_next0() if emit_next0 is not None else None
                av(15, e_prev)
                # copy accumulators (incl. denominator row) to SBUF right
                # away: frees both PSUM slots for the next pair's AVs
                oc = []
                for hh in range(2):
                    o_sb = oc_pool.tile([65, 512], bf16, tag="oc",
                                        name=f"oc_{qc}_{hp}_{hh}")
                    nc.vector.tensor_copy(out=o_sb, in_=ps_o[hh][0:65, :])
                    oc.append(o_sb)
                pending_norm.append((qc, hp, oc))
                return next_e

            def wo_dj(qc, dj):
                qsl = slice(qc * 512, (qc + 1) * 512)
                ps_y = ps_proj.tile([128, CW], f32, tag="pp",
                                    name=f"py_{qc}_{dj}")
                for kj in range(8):
                    mm(ps_y[:, 0:512], wo_sb[:, kj, dj * 128:(dj + 1) * 128],
                       OT_tiles[qc][:, kj, :], kj == 0, kj == 7)
                yt = y_pool.tile([128, 512], bf16, tag="yt",
                                 name=f"yt_{qc}_{dj}")
                if qc == 1:
                    # tail: ACT is idle after attention ends; bias-add there
                    nc.scalar.activation(
                        out=yt, in_=ps_y[:, 0:512],
                        func=AF.Identity, bias=bo_sb[:, dj:dj + 1])
                else:
                    nc.vector.tensor_scalar_add(
                        out=yt, in0=ps_y[:, 0:512], scalar1=bo_sb[:, dj:dj + 1])
                nc.sync.dma_start(
                    out=yT[dj * 128:(dj + 1) * 128, qsl], in_=yt)

            OT_tiles[0] = ot_pool.tile([128, 8, 512], bf16, tag="OT",
                                       name="OT_0")
            nc.sync.dma_start(out=wq_sb, in_=wq_r)
            for dj in range(8):
                q_proj_dj(0, dj)   # qc0 Q projection
            for tci in range(4):   # prefetch all xv chunks early
                xv_dma(tci)
            # Fine-grained PE fill-in inside qc0's ACT-bound head-pair loops:
            # one V-proj tile (~1.7us) or Q-proj dj block per kt slot, so the
            # QK->exp pipeline never starves for more than the 2-tile exp
            # buffer.
            # hp0 must produce V dc0 tiles at kt rate (its own AVs consume
            # them) and is PE-bound; spread the dc1 tiles over hp1-3 and the
            # qc1 Q-proj blocks over hp4-7 so the ACT-bound head-pairs stay
            # within their ~6us/hp PE fill-in budget.
            inj = {
                0: {kt: (lambda ti=kt: v_proj_tile(ti, 0)) for kt in range(16)},
                1: {kt: (lambda ti=i: v_proj_tile(ti, 1))
                    for i, kt in enumerate(range(0, 16, 3))},
                2: {kt: (lambda ti=6 + i: v_proj_tile(ti, 1))
                    for i, kt in enumerate(range(1, 15, 3))},
                3: {kt: (lambda ti=11 + i: v_proj_tile(ti, 1))
                    for i, kt in enumerate(range(2, 16, 3))},
            }
            for g in range(4):
                inj[4 + g] = {3: (lambda dj=2 * g: q_proj_dj(1, dj)),
                              11: (lambda dj=2 * g + 1: q_proj_dj(1, dj))}
            pre_e = None
            for hp in range(8):
                nxt = ((lambda h=hp + 1: qk_exp(0, h, 0)) if hp < 7
                       else (lambda: qk_exp(1, 0, 0)))
                pre_e = attn_hp(0, hp, inject=inj.get(hp), pre_e0=pre_e,
                                emit_next0=nxt)
            wo_sb = w_pool.tile([128, 8, D], bf16, tag="wbig", name="wo_sb")
            nc.sync.dma_start(out=wo_sb, in_=wo_r)
            OT_tiles[1] = ot_pool.tile([128, 8, 512], bf16, tag="OT",
                                       name="OT_1")
            for hp in range(8):
                nxt = (lambda h=hp + 1: qk_exp(1, h, 0)) if hp < 7 else None
                pre_e = attn_hp(1, hp, pre_e0=pre_e, emit_next0=nxt)
                if hp == 7:
                    # last pair's normalization before the final Wo block so
                    # its DVE chain hides under wo_dj(0,7)'s matmuls
                    flush_norm()
                wo_dj(0, hp)       # overlap qc0 output proj with qc1 attention
            for dj in range(8):
                wo_dj(1, dj)


def _xk8(kb):
    """[p, kj, j, t] fp8 pair layout: j0 = fp8(x), j1 = fp8(x/16)."""
    import ml_dtypes
    f8 = ml_dtypes.float8_e4m3
    xk = np.asarray(kb, np.float32).T            # [din, t]
    out = np.empty((128, 8, 2, xk.shape[1]), dtype=f8)
    out[:, :, 0, :] = xk.astype(f8).reshape(8, 128, -1).transpose(1, 0, 2)
    out[:, :, 1, :] = (xk / 16).astype(f8).reshape(8, 128, -1).transpose(1, 0, 2)
    return out


def _prep_inputs(query, key, value, Wq, bq, Wk, bk, Wv, bv, Wo, bo):
    import ml_dtypes
    bf = ml_dtypes.bfloat16

    query = np.asarray(query, np.float32)
    key = np.asarray(key, np.float32)
    value = np.asarray(value, np.float32)
    f8 = ml_dtypes.float8_e4m3
    wqT = np.ascontiguousarray(np.asarray(Wq, np.float32).T.astype(bf))
    WkT = np.asarray(Wk, np.float32).T           # [din, dout]
    wk_hi = WkT.astype(f8)
    wk_lo = ((WkT - wk_hi.astype(np.float32)) * 16).astype(f8)
    wk8 = np.empty((128, 8, 2, 1024), dtype=f8)  # [p, kj, j, dout]
    wk8[:, :, 0, :] = wk_hi.reshape(8, 128, 1024).transpose(1, 0, 2)
    wk8[:, :, 1, :] = wk_lo.reshape(8, 128, 1024).transpose(1, 0, 2)
    wvT = np.ascontiguousarray(np.asarray(Wv, np.float32).T.astype(bf))
    woT = np.ascontiguousarray(np.asarray(Wo, np.float32).T.astype(bf))
    bo_eff = np.asarray(bo, np.float32) + \
        np.asarray(Wo, np.float32) @ np.asarray(bv, np.float32)
    bq_t = np.ascontiguousarray(np.asarray(bq, np.float32).reshape(8, 128).T)
    bk_t = np.ascontiguousarray(np.asarray(bk, np.float32).reshape(8, 128).T)
    bo_t = np.ascontiguousarray(bo_eff.reshape(8, 128).T)

    in_maps = []
    for c in range(N_CORES):
        b, qh = c // 2, c % 2
        in_maps.append({
            "xqT": np.ascontiguousarray(
                query[b, qh * TQ:(qh + 1) * TQ, :].T.astype(bf)),
            "xk8": _xk8(key[b]),
            "xvT": np.ascontiguousarray(value[b].T.astype(bf)),
            "wqT": wqT, "wk8": wk8, "wvT": wvT, "woT": woT,
            "bq_in": bq_t, "bk_in": bk_t, "bo_in": bo_t,
        })
    return in_maps


def kernel(query, key, value, Wq, bq, Wk, bk, Wv, bv, Wo, bo):
    from concourse.bass_utils import run_bass_kernel_spmd

    if "nc" not in _CACHE:
        _CACHE["nc"] = _build_program()
    nc = _CACHE["nc"]

    in_maps = _prep_inputs(query, key, value, Wq, bq, Wk, bk, Wv, bv, Wo, bo)
    res = run_bass_kernel_spmd(nc, in_maps, list(range(N_CORES)))
    out = np.empty((B, S, D), np.float32)
    for c in range(N_CORES):
        b, qh = c // 2, c % 2
        out[b, qh * TQ:(qh + 1) * TQ, :] = \
            res.results[c]["yT"].T.astype(np.float32)
    return out



# revision 41
# speedup vs baseline: 1.1505x; 1.0323x over previous
"""Multi-head attention (B=4, S=2048, D=1024, H=16) on 8 trn2 NeuronCores.

Sharding: batch x query-half. Core c handles batch c//2, query rows
(c%2)*1024 : (c%2+1)*1024. Each core projects Q for its query chunk and K/V
for the full sequence of its batch (K/V projection duplicated across the two
cores sharing a batch), runs attention for all 16 heads, and applies the
output projection. No cross-core communication.

Device-side layout notes:
 - All activations are kept transposed ([feature, token]) so every matmul
   consumes operands directly: scores are computed as S^T[k,q] = K_h^T.T @ Q_h^T,
   softmax-exp runs on ScalarE, and the AV matmul contracts over k with
   lhsT = [V_h | ones-column] (stride 65), which makes row 64 of the PSUM
   output the softmax denominator. Normalization: DVE reciprocal + a 1x64
   PE matmul to replicate it across partitions + DVE multiply. No max
   subtraction (scores are O(+-5) here, fp32 exp is safe).
 - Head pairs (2h, 2h+1) sit in partitions 0:64 / 64:128 of the same tile.
 - Both heads' scores land in one [128,1024] PSUM tile -> a single ScalarE
   exp instruction, halving ACT instruction overhead.
 - fp8e4m3 + DoubleRow (0.5 PE cycles/row) for the two biggest matmul
   groups, with one-side residual correction to stay inside the 2e-2 gate:
   * QK: contraction pairs (K_hi, Q8) + (K_lo, Q8) where K_hi = fp8(K),
     K_lo = fp8(K - K_hi): K-side quantization error cancels, leaving only
     the Q-side fp8 error (~0.8e-2 end-to-end).
   * K projection: pairs (W_hi, x8) + (16*(W-W_hi), x8/16): weight-side
     error cancels. Host pre-quantizes wk8/xk8 in the paired layout.
   V/AV and Q/Wo projections stay bf16 (measured fp8 error there busts the
   tolerance). Scores softmax drops bk entirely: softmax over keys is
   invariant to the (Q+bq)@bk term (constant per query).
 - Phase order: K proj, Q proj(qc0), then attention; V-projection and
   Q(qc1) tiles are injected one-per-kt into the first head-pairs' loops
   (fine-grained so the QK->exp pipeline never starves), qc0's output
   projection interleaves into qc1's attention, and attention QK/exp run
   one slot ahead across head-pair boundaries (pre_e0/emit_next0 handoff).
 - V bias is folded into the output-projection bias host-side
   (softmax rows sum to 1 => attn @ (V + 1 b_v^T) = attn @ V + b_v^T).
 - y is staged/stored bf16 (host upcasts); ~1e-3 extra error, halves the
   output DMA.
"""

import numpy as np

B, S, D, H = 4, 2048, 1024, 16
DK = D // H          # 64
TQ = S // 2          # per-core query tokens
TK = S               # per-core key tokens
CW = 512             # x^T streaming chunk width (tokens)
N_CORES = 8
VP_W = H * (DK + 1)  # per head: 64 V columns + 1 ones column (stride 65)
SCALE = 1.0 / np.sqrt(DK)

_CACHE = {}


def _build_program(reps=1):
    import concourse.bass as bass
    import concourse.mybir as mybir
    from concourse import bacc
    from concourse.tile import TileContext

    f32 = mybir.dt.float32
    bf16 = mybir.dt.bfloat16
    AF = mybir.ActivationFunctionType

    nc = bacc.Bacc("TRN2", target_bir_lowering=False)

    fp8d = mybir.dt.float8e4
    xqT = nc.declare_dram_parameter("xqT", [D, TQ], bf16, isOutput=False)
    # K-proj operands pre-quantized host-side for fp8 DoubleRow:
    # xk8 [p, kj, j, t] with j0 = fp8(x), j1 = fp8(x/16);
    # wk8 [p, kj, j, dout] with j0 = W_hi, j1 = 16*(W - W_hi).
    xk8 = nc.declare_dram_parameter("xk8", [128, 8, 2, TK], fp8d,
                                    isOutput=False)
    xvT = nc.declare_dram_parameter("xvT", [D, TK], bf16, isOutput=False)
    wqT = nc.declare_dram_parameter("wqT", [D, D], bf16, isOutput=False)
    wk8 = nc.declare_dram_parameter("wk8", [128, 8, 2, D], fp8d,
                                    isOutput=False)
    wvT = nc.declare_dram_parameter("wvT", [D, D], bf16, isOutput=False)
    woT = nc.declare_dram_parameter("woT", [D, D], bf16, isOutput=False)
    bq_in = nc.declare_dram_parameter("bq_in", [128, 8], f32, isOutput=False)
    bk_in = nc.declare_dram_parameter("bk_in", [128, 8], f32, isOutput=False)
    bo_in = nc.declare_dram_parameter("bo_in", [128, 8], f32, isOutput=False)
    yT = nc.declare_dram_parameter("yT", [D, TQ], bf16, isOutput=True)

    # DRAM access helpers: feature dim split as (tile j, partition p)
    xq_r = xqT[:].rearrange("(a p) t -> p a t", p=128)
    xk_r = xk8[:]
    xv_r = xvT[:].rearrange("(a p) t -> p a t", p=128)
    wq_r = wqT[:].rearrange("(a p) d -> p a d", p=128)
    wk_r = wk8[:]
    wv_r = wvT[:].rearrange("(a p) d -> p a d", p=128)
    wo_r = woT[:].rearrange("(a p) d -> p a d", p=128)

    with TileContext(nc) as tc:
        for _rep in range(reps):
            _emit_body(nc, tc, bass, f32, bf16, AF,
                       xq_r, xk_r, xv_r, wq_r, wk_r, wv_r, wo_r,
                       bq_in, bk_in, bo_in, yT)
    nc.compile()
    return nc


def _emit_body(nc, tc, bass, f32, bf16, AF,
               xq_r, xk_r, xv_r, wq_r, wk_r, wv_r, wo_r,
               bq_in, bk_in, bo_in, yT):
    import concourse.mybir as mybir
    def mm(out, lhsT, rhs, start, stop):
        nc.tensor.matmul(out, lhsT=lhsT, rhs=rhs, start=start, stop=stop)

    if True:
        with (
            tc.tile_pool(name="const", bufs=1) as const_pool,
            tc.tile_pool(name="kt_res", bufs=1) as kt_pool,
            tc.tile_pool(name="qt_res", bufs=1) as qt_pool,
            tc.tile_pool(name="vp_res", bufs=1) as vp_pool,
            tc.tile_pool(name="ot_res", bufs=2) as ot_pool,
            tc.tile_pool(name="w_res", bufs=2) as w_pool,
            tc.tile_pool(name="x_str", bufs=2) as x_pool,
            tc.tile_pool(name="exp_p", bufs=3) as exp_pool,
            tc.tile_pool(name="rec_p", bufs=1) as rec_pool,
            tc.tile_pool(name="recb_p", bufs=2) as recb_pool,
            tc.tile_pool(name="oc_p", bufs=4) as oc_pool,
            tc.tile_pool(name="y_p", bufs=2) as y_pool,
        ):
            # Phase-scoped PSUM pools: projections get a deep 4-buffer pool
            # while the attention pools' banks are still free, so the PE
            # never waits on the ACT-copy -> DVE-sub evacuation round trip.
            ps_proj = tc.alloc_tile_pool(name="ps_projA", bufs=4,
                                         space="PSUM")
            PSP = [ps_proj]
            PSS = [None]
            PSA = [None]
            bq_sb = const_pool.tile([128, 8], f32, tag="bq")
            bk_sb = const_pool.tile([128, 8], f32, tag="bk")
            bo_sb = const_pool.tile([128, 8], f32, tag="bo")
            # separate queue: keep these tiny loads off the head of the
            # sync queue that feeds the first matmuls
            nc.gpsimd.dma_start(out=bq_sb, in_=bq_in[:])
            nc.gpsimd.dma_start(out=bk_sb, in_=bk_in[:])
            nc.gpsimd.dma_start(out=bo_sb, in_=bo_in[:])

            fp8 = mybir.dt.float8e4
            # [p, hp, j, t]: j=0 -> K_hi / Q8, j=1 -> K_lo residual / Q8 dup
            KT_sb = kt_pool.tile([128, 8, 2, TK], fp8, tag="KT")
            QT_sb = qt_pool.tile([128, 8, 2, TQ], fp8, tag="QT")
            Vp_sb = vp_pool.tile([128, 16, VP_W], bf16, tag="Vp")  # [p, i, c]
            # view: [p, ktile, head, col(65)]
            Vp4 = Vp_sb.rearrange("p i (hh c) -> p i hh c", c=DK + 1)
            nc.vector.memset(Vp4[:, :, :, DK], 1.0)
            ones_sb = const_pool.tile([1, 64], bf16, tag="ones")
            nc.vector.memset(ones_sb, 1.0)

            # ---- K projection: K^T[dout, t], streamed x chunks ----
            # split loads so the first dj-column's matmuls start early
            # K projection in fp8 DoubleRow: pairs (W_hi, x8) + (W_lo*16,
            # x8/16) -> W-side quantization corrected, x-side ~fp8. 0.5
            # cycles/row => K proj at half bf16 PE cost.
            wq_sb = w_pool.tile([128, 8, D], bf16, tag="wbig", name="wq_sb")
            xq_tiles = {}
            wk_sb = w_pool.tile([128, 8, 2, D], fp8, tag="wk8", name="wk_sb",
                                bufs=1)
            def q_proj_dj(tci, dj):
                if dj == 0:
                    xq_tiles[tci] = x_pool.tile([128, 8, CW], bf16,
                                                tag="xchunk",
                                                name=f"xq_{tci}")
                    nc.sync.dma_start(
                        out=xq_tiles[tci],
                        in_=xq_r[:, :, tci * CW:(tci + 1) * CW])
                xc = xq_tiles[tci]
                ps = PSP[0].tile([128, CW], f32, tag="pp",
                                  name=f"pq_{tci}_{dj}")
                for kj in range(8):
                    mm(ps, wq_sb[:, kj, dj * 128:(dj + 1) * 128],
                       xc[:, kj, :], kj == 0, kj == 7)
                qsl = slice(tci * CW, (tci + 1) * CW)
                nc.vector.tensor_scalar_add(
                    out=QT_sb[:, dj, 0, qsl], in0=ps,
                    scalar1=bq_sb[:, dj:dj + 1])
                # duplicate Q8 into the second DoubleRow slot (pairs with
                # K_lo); cheap 16-bit-view copy
                nc.vector.tensor_copy(
                    out=QT_sb[:, dj, 1, qsl].bitcast(mybir.dt.uint16),
                    in_=QT_sb[:, dj, 0, qsl].bitcast(mybir.dt.uint16))

            nc.sync.dma_start(out=wk_sb[:, :, :, 0:128],
                              in_=wk_r[:, :, :, 0:128])
            for tci in range(TK // CW):
                xc = x_pool.tile([128, 8, 2, CW], fp8, tag="xchunk",
                                 name=f"xk_{tci}")
                if tci == 0:
                    for kq in range(4):
                        nc.sync.dma_start(
                            out=xc[:, 2 * kq:2 * kq + 2, :, :],
                            in_=xk_r[:, 2 * kq:2 * kq + 2, :, 0:CW])
                    # remaining K-weight columns, one dj-slice each, queued
                    # behind the first activation chunk
                    for djw in range(1, 8):
                        nc.sync.dma_start(
                            out=wk_sb[:, :, :, djw * 128:(djw + 1) * 128],
                            in_=wk_r[:, :, :, djw * 128:(djw + 1) * 128])
                else:
                    nc.sync.dma_start(
                        out=xc, in_=xk_r[:, :, :, tci * CW:(tci + 1) * CW])
                for dj in range(8):
                    ps = PSP[0].tile([128, CW], f32, tag="pp",
                                      name=f"pk_{tci}_{dj}")
                    for kj in range(8):
                        nc.tensor.matmul(
                            ps, lhsT=wk_sb[:, kj, :, dj * 128:(dj + 1) * 128],
                            rhs=xc[:, kj, :, :], start=kj == 0, stop=kj == 7,
                            perf_mode=mybir.MatmulPerfMode.DoubleRow)
                    # bk is dropped: softmax over keys is invariant to the
                    # (Q+bq)@bk term (constant per query). K_hi = fp8(K),
                    # K_lo = fp8(K - K_hi) -> DoubleRow pair corrects the
                    # K-side quantization error.
                    ksl = slice(tci * CW, (tci + 1) * CW)
                    # K_hi evac on ScalarE: it is idle during the projection
                    # phase, and this halves the DVE chain that was binding it
                    nc.scalar.activation(out=KT_sb[:, dj, 0, ksl], in_=ps,
                                         func=AF.Copy)
                    with nc.allow_low_precision(
                            reason="fp8 residual capture for DoubleRow QK"):
                        nc.vector.tensor_sub(out=KT_sb[:, dj, 1, ksl],
                                             in0=ps, in1=KT_sb[:, dj, 0, ksl])


            # V and Wo weights resident; V projection is fused into the first
            # head-pair's attention loop below so ScalarE exp overlaps it
            wv_sb = w_pool.tile([128, 8, D], bf16, tag="wbig", name="wv_sb")
            nc.sync.dma_start(out=wv_sb, in_=wv_r)
            wo_sb = None  # loaded just before qc1 (reuses wq's buffer)

            xv_tiles = {}

            def xv_dma(tci):
                xc = x_pool.tile([128, 8, CW], bf16, tag="xvchunk",
                                 name=f"xv_{tci}", bufs=4)
                nc.sync.dma_start(out=xc,
                                  in_=xv_r[:, :, tci * CW:(tci + 1) * CW])
                xv_tiles[tci] = xc

            def v_proj_tile(ti, dc):
                xc = xv_tiles[ti // 4]
                ts2 = ti % 4
                ps = PSP[0].tile([128, CW], f32, tag="pp",
                                  name=f"pv_{ti}_{dc}")
                for kj in range(8):
                    mm(ps[:, 0:512], xc[:, kj, ts2 * 128:(ts2 + 1) * 128],
                       wv_sb[:, kj, dc * 512:(dc + 1) * 512],
                       kj == 0, kj == 7)
                nc.vector.tensor_copy(
                    out=Vp4[:, ti, dc * 8:(dc + 1) * 8, 0:DK],
                    in_=ps[:, 0:512].rearrange("p (hh c) -> p hh c", c=DK))

            OT_tiles = {}

            # deferred normalization: (qc, hp, sbuf copy of [65,512] accum)
            pending_norm = []

            def flush_norm():
                while pending_norm:
                    qc, hp, oc = pending_norm.pop(0)
                    for hh in range(2):
                        # row 64 of oc = softmax denominator
                        rec = rec_pool.tile([1, 512], bf16, tag="rec",
                                            name=f"rec_{qc}_{hp}_{hh}")
                        with nc.allow_low_precision(
                                reason="softmax denom reciprocal, bf16 "
                                       "matches pipeline precision"):
                            nc.vector.reciprocal(out=rec,
                                                 in_=oc[hh][64:65, :])
                        # replicate reciprocal across 64 partitions via PE
                        ps_rep = PSP[0].tile([128, CW], f32, tag="pp",
                                              name=f"pr_{qc}_{hp}_{hh}")
                        mm(ps_rep[0:64, 0:512], ones_sb, rec, True, True)
                        recb = recb_pool.tile([64, 512], bf16, tag="recb",
                                              name=f"recb_{qc}_{hp}_{hh}")
                        nc.vector.tensor_copy(out=recb,
                                              in_=ps_rep[0:64, 0:512])
                        nc.vector.tensor_mul(
                            out=OT_tiles[qc][hh * 64:(hh + 1) * 64, hp, :],
                            in0=oc[hh][0:64, :], in1=recb)

            def qk_exp(qc, hp, kt):
                # both heads' scores^T into one 2-bank PSUM tile.
                # fp8 DoubleRow: contraction pairs (K_hi, Q8) + (K_lo, Q8)
                # -> (K_hi + K_lo)^T @ Q8 with K-side quantization error
                # corrected; charged 0.5 cycles/row by the PE.
                qsl = slice(qc * 512, (qc + 1) * 512)
                pss = PSS[0].tile([128, 1024], f32, tag="pss",
                                name=f"pss_{qc}_{hp}_{kt}")
                for hh in range(2):
                    pb = hh * 64
                    nc.tensor.matmul(
                        pss[:, hh * 512:(hh + 1) * 512],
                        lhsT=KT_sb[pb:pb + 64, hp, :,
                                   kt * 128:(kt + 1) * 128],
                        rhs=QT_sb[pb:pb + 64, hp, :, qsl],
                        start=True, stop=True,
                        perf_mode=mybir.MatmulPerfMode.DoubleRow)
                e = exp_pool.tile([128, 1024], bf16, tag="ex",
                                  name=f"ex_{qc}_{hp}_{kt}")
                nc.scalar.activation(out=e, in_=pss, func=AF.Exp,
                                     scale=SCALE)
                return e

            def attn_hp(qc, hp, inject=None, pre_e0=None, emit_next0=None):
                ps_o = [PSA[0].tile([128, 512], f32, tag="po",
                                   name=f"po_{qc}_{hp}_{i}")
                        for i in range(2)]

                def av(kt, e):
                    for hh in range(2):
                        h = 2 * hp + hh
                        mm(ps_o[hh][0:65, :],
                           Vp_sb[:, kt, 65 * h:65 * h + 65],
                           e[:, hh * 512:(hh + 1) * 512],
                           kt == 0, kt == 15)

                # QK/exp run one kt ahead of AV (and one slot across the
                # head-pair boundary via pre_e0/emit_next0) so ScalarE never
                # waits for the boundary accumulator handoff.
                e_prev = pre_e0
                kt0 = 0
                if pre_e0 is not None:
                    kt0 = 1
                    if inject is not None and inject.get(0) is not None:
                        inject[0]()
                for kt in range(kt0, 16):
                    if inject is not None and inject.get(kt) is not None:
                        inject[kt]()   # fine-grained PE fill-in (~1.7us max)
                    if kt == 2:
                        # previous pair's normalization, now off the
                        # critical path
                        flush_norm()
                    e = qk_exp(qc, hp, kt)
                    if e_prev is not None:
                        av(kt - 1, e_prev)
                    e_prev = e
                next_e = emit_next0() if emit_next0 is not None else None
                av(15, e_prev)
                # copy accumulators (incl. denominator row) to SBUF right
                # away: frees both PSUM slots for the next pair's AVs
                oc = []
                for hh in range(2):
                    o_sb = oc_pool.tile([65, 512], bf16, tag="oc",
                                        name=f"oc_{qc}_{hp}_{hh}")
                    nc.vector.tensor_copy(out=o_sb, in_=ps_o[hh][0:65, :])
                    oc.append(o_sb)
                pending_norm.append((qc, hp, oc))
                return next_e

            def wo_dj(qc, dj):
                qsl = slice(qc * 512, (qc + 1) * 512)
                ps_y = PSP[0].tile([128, CW], f32, tag="pp",
                                    name=f"py_{qc}_{dj}")
                for kj in range(8):
                    mm(ps_y[:, 0:512], wo_sb[:, kj, dj * 128:(dj + 1) * 128],
                       OT_tiles[qc][:, kj, :], kj == 0, kj == 7)
                yt = y_pool.tile([128, 512], bf16, tag="yt",
                                 name=f"yt_{qc}_{dj}")
                if qc == 1:
                    # tail: ACT is idle after attention ends; bias-add there
                    nc.scalar.activation(
                        out=yt, in_=ps_y[:, 0:512],
                        func=AF.Identity, bias=bo_sb[:, dj:dj + 1])
                else:
                    nc.vector.tensor_scalar_add(
                        out=yt, in0=ps_y[:, 0:512], scalar1=bo_sb[:, dj:dj + 1])
                nc.sync.dma_start(
                    out=yT[dj * 128:(dj + 1) * 128, qsl], in_=yt)

            OT_tiles[0] = ot_pool.tile([128, 8, 512], bf16, tag="OT",
                                       name="OT_0")
            nc.sync.dma_start(out=wq_sb, in_=wq_r)
            q_proj_dj(0, 0)   # attention head-pair 0 only needs Q dj0;
            q_proj_dj(0, 1)   # dj1 for the hp0->hp1 handoff. djs 2-7 are
            ps_proj.release()  # injected into hp0-5 below.
            PSP[0] = tc.alloc_tile_pool(name="ps_proj", bufs=2, space="PSUM")
            PSS[0] = tc.alloc_tile_pool(name="ps_s", bufs=2, space="PSUM")
            PSA[0] = tc.alloc_tile_pool(name="ps_av", bufs=2, space="PSUM")
            for tci in range(4):   # prefetch all xv chunks early
                xv_dma(tci)
            # Fine-grained PE fill-in inside qc0's ACT-bound head-pair loops:
            # one V-proj tile (~1.7us) or Q-proj dj block per kt slot, so the
            # QK->exp pipeline never starves for more than the 2-tile exp
            # buffer.
            # hp0 must produce V dc0 tiles at kt rate (its own AVs consume
            # them) and is PE-bound; spread the dc1 tiles over hp1-3 and the
            # qc1 Q-proj blocks over hp4-7 so the ACT-bound head-pairs stay
            # within their ~6us/hp PE fill-in budget.
            inj = {
                0: {kt: (lambda ti=kt: v_proj_tile(ti, 0)) for kt in range(16)},
                1: {kt: (lambda ti=i: v_proj_tile(ti, 1))
                    for i, kt in enumerate(range(0, 16, 3))},
                2: {kt: (lambda ti=6 + i: v_proj_tile(ti, 1))
                    for i, kt in enumerate(range(1, 15, 3))},
                3: {kt: (lambda ti=11 + i: v_proj_tile(ti, 1))
                    for i, kt in enumerate(range(2, 16, 3))},
            }
            for g in range(4):
                inj[4 + g] = {3: (lambda dj=2 * g: q_proj_dj(1, dj)),
                              11: (lambda dj=2 * g + 1: q_proj_dj(1, dj))}
            # qc0 Q-proj djs 2-7 staggered so QT[dj=h+1] is ready before
            # head-pair h's end-of-loop handoff emits qk(0, h+1, 0)
            for h in range(6):
                inj[h][13] = (lambda a=inj[h].get(13), dj=h + 2:
                              (a() if a else None, q_proj_dj(0, dj))[-1])
            pre_e = None
            for hp in range(8):
                nxt = ((lambda h=hp + 1: qk_exp(0, h, 0)) if hp < 7
                       else (lambda: qk_exp(1, 0, 0)))
                pre_e = attn_hp(0, hp, inject=inj.get(hp), pre_e0=pre_e,
                                emit_next0=nxt)
            wo_sb = w_pool.tile([128, 8, D], bf16, tag="wbig", name="wo_sb")
            nc.sync.dma_start(out=wo_sb, in_=wo_r)
            OT_tiles[1] = ot_pool.tile([128, 8, 512], bf16, tag="OT",
                                       name="OT_1")
            for hp in range(8):
                nxt = (lambda h=hp + 1: qk_exp(1, h, 0)) if hp < 7 else None
                pre_e = attn_hp(1, hp, pre_e0=pre_e, emit_next0=nxt)
                if hp == 7:
                    # last pair's normalization before the final Wo block so
                    # its DVE chain hides under wo_dj(0,7)'s matmuls
                    flush_norm()
                wo_dj(0, hp)       # overlap qc0 output proj with qc1 attention
            for dj in range(8):
                wo_dj(1, dj)
            PSA[0].release()
            PSS[0].release()
            PSP[0].release()


def _xk8(kb):
    """[p, kj, j, t] fp8 pair layout: j0 = fp8(x), j1 = fp8(x/16)."""
    import ml_dtypes
    f8 = ml_dtypes.float8_e4m3
    xk = np.asarray(kb, np.float32).T            # [din, t]
    out = np.empty((128, 8, 2, xk.shape[1]), dtype=f8)
    out[:, :, 0, :] = xk.astype(f8).reshape(8, 128, -1).transpose(1, 0, 2)
    out[:, :, 1, :] = (xk / 16).astype(f8).reshape(8, 128, -1).transpose(1, 0, 2)
    return out


def _prep_inputs(query, key, value, Wq, bq, Wk, bk, Wv, bv, Wo, bo):
    import ml_dtypes
    bf = ml_dtypes.bfloat16

    query = np.asarray(query, np.float32)
    key = np.asarray(key, np.float32)
    value = np.asarray(value, np.float32)
    f8 = ml_dtypes.float8_e4m3
    wqT = np.ascontiguousarray(np.asarray(Wq, np.float32).T.astype(bf))
    WkT = np.asarray(Wk, np.float32).T           # [din, dout]
    wk_hi = WkT.astype(f8)
    wk_lo = ((WkT - wk_hi.astype(np.float32)) * 16).astype(f8)
    wk8 = np.empty((128, 8, 2, 1024), dtype=f8)  # [p, kj, j, dout]
    wk8[:, :, 0, :] = wk_hi.reshape(8, 128, 1024).transpose(1, 0, 2)
    wk8[:, :, 1, :] = wk_lo.reshape(8, 128, 1024).transpose(1, 0, 2)
    wvT = np.ascontiguousarray(np.asarray(Wv, np.float32).T.astype(bf))
    woT = np.ascontiguousarray(np.asarray(Wo, np.float32).T.astype(bf))
    bo_eff = np.asarray(bo, np.float32) + \
        np.asarray(Wo, np.float32) @ np.asarray(bv, np.float32)
    bq_t = np.ascontiguousarray(np.asarray(bq, np.float32).reshape(8, 128).T)
    bk_t = np.ascontiguousarray(np.asarray(bk, np.float32).reshape(8, 128).T)
    bo_t = np.ascontiguousarray(bo_eff.reshape(8, 128).T)

    in_maps = []
    for c in range(N_CORES):
        b, qh = c // 2, c % 2
        in_maps.append({
            "xqT": np.ascontiguousarray(
                query[b, qh * TQ:(qh + 1) * TQ, :].T.astype(bf)),
            "xk8": _xk8(key[b]),
            "xvT": np.ascontiguousarray(value[b].T.astype(bf)),
            "wqT": wqT, "wk8": wk8, "wvT": wvT, "woT": woT,
            "bq_in": bq_t, "bk_in": bk_t, "bo_in": bo_t,
        })
    return in_maps


def kernel(query, key, value, Wq, bq, Wk, bk, Wv, bv, Wo, bo):
    from concourse.bass_utils import run_bass_kernel_spmd

    if "nc" not in _CACHE:
        _CACHE["nc"] = _build_program()
    nc = _CACHE["nc"]

    in_maps = _prep_inputs(query, key, value, Wq, bq, Wk, bk, Wv, bv, Wo, bo)
    res = run_bass_kernel_spmd(nc, in_maps, list(range(N_CORES)))
    out = np.empty((B, S, D), np.float32)
    for c in range(N_CORES):
        b, qh = c // 2, c % 2
        out[b, qh * TQ:(qh + 1) * TQ, :] = \
            res.results[c]["yT"].T.astype(np.float32)
    return out



# revision 43
# speedup vs baseline: 1.1995x; 1.0426x over previous
"""Multi-head attention (B=4, S=2048, D=1024, H=16) on 8 trn2 NeuronCores.

Sharding: batch x query-half. Core c handles batch c//2, query rows
(c%2)*1024 : (c%2+1)*1024. Each core projects Q for its query chunk and K/V
for the full sequence of its batch (K/V projection duplicated across the two
cores sharing a batch), runs attention for all 16 heads, and applies the
output projection. No cross-core communication.

Device-side layout notes:
 - All activations are kept transposed ([feature, token]) so every matmul
   consumes operands directly: scores are computed as S^T[k,q] = K_h^T.T @ Q_h^T,
   softmax-exp runs on ScalarE, and the AV matmul contracts over k with
   lhsT = [V_h | ones-column] (stride 65), which makes row 64 of the PSUM
   output the softmax denominator. Normalization: DVE reciprocal + a 1x64
   PE matmul to replicate it across partitions + DVE multiply. No max
   subtraction (scores are O(+-5) here, fp32 exp is safe).
 - Head pairs (2h, 2h+1) sit in partitions 0:64 / 64:128 of the same tile.
 - Both heads' scores land in one [128,1024] PSUM tile -> a single ScalarE
   exp instruction, halving ACT instruction overhead.
 - fp8e4m3 + DoubleRow (0.5 PE cycles/row) for the two biggest matmul
   groups, with one-side residual correction to stay inside the 2e-2 gate:
   * QK: contraction pairs (K_hi, Q8) + (K_lo, Q8) where K_hi = fp8(K),
     K_lo = fp8(K - K_hi): K-side quantization error cancels, leaving only
     the Q-side fp8 error (~0.8e-2 end-to-end).
   * K projection: pairs (W_hi, x8) + (16*(W-W_hi), x8/16): weight-side
     error cancels. Host pre-quantizes wk8/xk8 in the paired layout.
   V/AV and Q/Wo projections stay bf16 (measured fp8 error there busts the
   tolerance). Scores softmax drops bk entirely: softmax over keys is
   invariant to the (Q+bq)@bk term (constant per query).
 - Phase order: K proj, Q proj(qc0), then attention; V-projection and
   Q(qc1) tiles are injected one-per-kt into the first head-pairs' loops
   (fine-grained so the QK->exp pipeline never starves), qc0's output
   projection interleaves into qc1's attention, and attention QK/exp run
   one slot ahead across head-pair boundaries (pre_e0/emit_next0 handoff).
 - V bias is folded into the output-projection bias host-side
   (softmax rows sum to 1 => attn @ (V + 1 b_v^T) = attn @ V + b_v^T).
 - y is staged/stored bf16 (host upcasts); ~1e-3 extra error, halves the
   output DMA.
"""

import numpy as np

B, S, D, H = 4, 2048, 1024, 16
DK = D // H          # 64
TQ = S // 2          # per-core query tokens
TK = S               # per-core key tokens
CW = 512             # x^T streaming chunk width (tokens)
N_CORES = 8
VP_W = H * (DK + 1)  # per head: 64 V columns + 1 ones column (stride 65)
SCALE = 1.0 / np.sqrt(DK)

_CACHE = {}


def _build_program(reps=1):
    import concourse.bass as bass
    import concourse.mybir as mybir
    from concourse import bacc
    from concourse.tile import TileContext

    f32 = mybir.dt.float32
    bf16 = mybir.dt.bfloat16
    AF = mybir.ActivationFunctionType

    nc = bacc.Bacc("TRN2", target_bir_lowering=False)

    fp8d = mybir.dt.float8e4
    xqT = nc.declare_dram_parameter("xqT", [D, TQ], bf16, isOutput=False)
    # K-proj operands pre-quantized host-side for fp8 DoubleRow:
    # xk8 [p, kj, j, t] with j0 = fp8(x), j1 = fp8(x/16);
    # wk8 [p, kj, j, dout] with j0 = W_hi, j1 = 16*(W - W_hi).
    xk8 = nc.declare_dram_parameter("xk8", [128, 8, 2, TK], fp8d,
                                    isOutput=False)
    xvT = nc.declare_dram_parameter("xvT", [D, TK], bf16, isOutput=False)
    wqT = nc.declare_dram_parameter("wqT", [D, D], bf16, isOutput=False)
    wk8 = nc.declare_dram_parameter("wk8", [128, 8, 2, D], fp8d,
                                    isOutput=False)
    wvT = nc.declare_dram_parameter("wvT", [D, D], bf16, isOutput=False)
    woT = nc.declare_dram_parameter("woT", [D, D], bf16, isOutput=False)
    bq_in = nc.declare_dram_parameter("bq_in", [128, 8], f32, isOutput=False)
    bk_in = nc.declare_dram_parameter("bk_in", [128, 8], f32, isOutput=False)
    bo_in = nc.declare_dram_parameter("bo_in", [128, 8], f32, isOutput=False)
    yT = nc.declare_dram_parameter("yT", [D, TQ], bf16, isOutput=True)

    # DRAM access helpers: feature dim split as (tile j, partition p)
    xq_r = xqT[:].rearrange("(a p) t -> p a t", p=128)
    xk_r = xk8[:]
    xv_r = xvT[:].rearrange("(a p) t -> p a t", p=128)
    wq_r = wqT[:].rearrange("(a p) d -> p a d", p=128)
    wk_r = wk8[:]
    wv_r = wvT[:].rearrange("(a p) d -> p a d", p=128)
    wo_r = woT[:].rearrange("(a p) d -> p a d", p=128)

    with TileContext(nc) as tc:
        for _rep in range(reps):
            _emit_body(nc, tc, bass, f32, bf16, AF,
                       xq_r, xk_r, xv_r, wq_r, wk_r, wv_r, wo_r,
                       bq_in, bk_in, bo_in, yT)
    nc.compile()
    return nc


def _emit_body(nc, tc, bass, f32, bf16, AF,
               xq_r, xk_r, xv_r, wq_r, wk_r, wv_r, wo_r,
               bq_in, bk_in, bo_in, yT):
    import concourse.mybir as mybir
    def mm(out, lhsT, rhs, start, stop):
        nc.tensor.matmul(out, lhsT=lhsT, rhs=rhs, start=start, stop=stop)

    if True:
        with (
            tc.tile_pool(name="const", bufs=1) as const_pool,
            tc.tile_pool(name="kt_res", bufs=1) as kt_pool,
            tc.tile_pool(name="qt_res", bufs=1) as qt_pool,
            tc.tile_pool(name="vp_res", bufs=1) as vp_pool,
            tc.tile_pool(name="ot_res", bufs=2) as ot_pool,
            tc.tile_pool(name="w_res", bufs=2) as w_pool,
            tc.tile_pool(name="x_str", bufs=2) as x_pool,
            tc.tile_pool(name="exp_p", bufs=3) as exp_pool,
            tc.tile_pool(name="rec_p", bufs=1) as rec_pool,
            tc.tile_pool(name="recb_p", bufs=2) as recb_pool,
            tc.tile_pool(name="oc_p", bufs=4) as oc_pool,
            tc.tile_pool(name="y_p", bufs=2) as y_pool,
        ):
            # Phase-scoped PSUM pools: projections get a deep 4-buffer pool
            # while the attention pools' banks are still free, so the PE
            # never waits on the ACT-copy -> DVE-sub evacuation round trip.
            ps_proj = tc.alloc_tile_pool(name="ps_projA", bufs=4,
                                         space="PSUM")
            PSP = [ps_proj]
            PSS = [None]
            PSA = [None]
            bq_sb = const_pool.tile([128, 8], f32, tag="bq")
            bk_sb = const_pool.tile([128, 8], f32, tag="bk")
            bo_sb = const_pool.tile([128, 8], f32, tag="bo")
            # separate queue: keep these tiny loads off the head of the
            # sync queue that feeds the first matmuls
            nc.gpsimd.dma_start(out=bq_sb, in_=bq_in[:])
            nc.gpsimd.dma_start(out=bk_sb, in_=bk_in[:])
            nc.gpsimd.dma_start(out=bo_sb, in_=bo_in[:])

            fp8 = mybir.dt.float8e4
            # [p, hp, j, t]: j=0 -> K_hi / Q8, j=1 -> K_lo residual / Q8 dup
            KT_sb = kt_pool.tile([128, 8, 2, TK], fp8, tag="KT")
            QT_sb = qt_pool.tile([128, 8, 2, TQ], fp8, tag="QT")
            Vp_sb = vp_pool.tile([128, 16, VP_W], bf16, tag="Vp")  # [p, i, c]
            # view: [p, ktile, head, col(65)]
            Vp4 = Vp_sb.rearrange("p i (hh c) -> p i hh c", c=DK + 1)
            nc.vector.memset(Vp4[:, :, :, DK], 1.0)
            ones_sb = const_pool.tile([1, 64], bf16, tag="ones")
            nc.vector.memset(ones_sb, 1.0)

            # ---- K projection: K^T[dout, t], streamed x chunks ----
            # split loads so the first dj-column's matmuls start early
            # K projection in fp8 DoubleRow: pairs (W_hi, x8) + (W_lo*16,
            # x8/16) -> W-side quantization corrected, x-side ~fp8. 0.5
            # cycles/row => K proj at half bf16 PE cost.
            wq_sb = w_pool.tile([128, 8, D], bf16, tag="wbig", name="wq_sb")
            xq_tiles = {}
            wk_sb = w_pool.tile([128, 8, 2, D], fp8, tag="wk8", name="wk_sb",
                                bufs=1)
            def q_proj_dj(tci, dj):
                if dj == 0:
                    xq_tiles[tci] = x_pool.tile([128, 8, CW], bf16,
                                                tag="xchunk",
                                                name=f"xq_{tci}")
                    nc.sync.dma_start(
                        out=xq_tiles[tci],
                        in_=xq_r[:, :, tci * CW:(tci + 1) * CW])
                xc = xq_tiles[tci]
                ps = PSP[0].tile([128, CW], f32, tag="pp",
                                  name=f"pq_{tci}_{dj}")
                for kj in range(8):
                    mm(ps, wq_sb[:, kj, dj * 128:(dj + 1) * 128],
                       xc[:, kj, :], kj == 0, kj == 7)
                qsl = slice(tci * CW, (tci + 1) * CW)
                nc.vector.tensor_scalar_add(
                    out=QT_sb[:, dj, 0, qsl], in0=ps,
                    scalar1=bq_sb[:, dj:dj + 1])
                # duplicate Q8 into the second DoubleRow slot (pairs with
                # K_lo); cheap 16-bit-view copy
                nc.vector.tensor_copy(
                    out=QT_sb[:, dj, 1, qsl].bitcast(mybir.dt.uint16),
                    in_=QT_sb[:, dj, 0, qsl].bitcast(mybir.dt.uint16))

            nc.sync.dma_start(out=wk_sb[:, :, :, 0:128],
                              in_=wk_r[:, :, :, 0:128])
            for tci in range(TK // CW):
                xc = x_pool.tile([128, 8, 2, CW], fp8, tag="xchunk",
                                 name=f"xk_{tci}")
                if tci == 0:
                    for kq in range(4):
                        nc.sync.dma_start(
                            out=xc[:, 2 * kq:2 * kq + 2, :, :],
                            in_=xk_r[:, 2 * kq:2 * kq + 2, :, 0:CW])
                    # remaining K-weight columns, one dj-slice each, queued
                    # behind the first activation chunk
                    for djw in range(1, 8):
                        nc.sync.dma_start(
                            out=wk_sb[:, :, :, djw * 128:(djw + 1) * 128],
                            in_=wk_r[:, :, :, djw * 128:(djw + 1) * 128])
                else:
                    nc.sync.dma_start(
                        out=xc, in_=xk_r[:, :, :, tci * CW:(tci + 1) * CW])
                for dj in range(8):
                    ps = PSP[0].tile([128, CW], f32, tag="pp",
                                      name=f"pk_{tci}_{dj}")
                    for kj in range(8):
                        nc.tensor.matmul(
                            ps, lhsT=wk_sb[:, kj, :, dj * 128:(dj + 1) * 128],
                            rhs=xc[:, kj, :, :], start=kj == 0, stop=kj == 7,
                            perf_mode=mybir.MatmulPerfMode.DoubleRow)
                    # bk is dropped: softmax over keys is invariant to the
                    # (Q+bq)@bk term (constant per query). K_hi = fp8(K),
                    # K_lo = fp8(K - K_hi) -> DoubleRow pair corrects the
                    # K-side quantization error.
                    ksl = slice(tci * CW, (tci + 1) * CW)
                    # K_hi evac on ScalarE: it is idle during the projection
                    # phase, and this halves the DVE chain that was binding it
                    nc.scalar.activation(out=KT_sb[:, dj, 0, ksl], in_=ps,
                                         func=AF.Copy)
                    with nc.allow_low_precision(
                            reason="fp8 residual capture for DoubleRow QK"):
                        nc.vector.tensor_sub(out=KT_sb[:, dj, 1, ksl],
                                             in0=ps, in1=KT_sb[:, dj, 0, ksl])


            # V and Wo weights resident; V projection is fused into the first
            # head-pair's attention loop below so ScalarE exp overlaps it
            wv_sb = w_pool.tile([128, 8, D], bf16, tag="wbig", name="wv_sb")
            nc.sync.dma_start(out=wv_sb, in_=wv_r)
            wo_sb = None  # loaded just before qc1 (reuses wq's buffer)

            xv_tiles = {}

            def xv_dma(tci):
                xc = x_pool.tile([128, 8, CW], bf16, tag="xvchunk",
                                 name=f"xv_{tci}", bufs=4)
                nc.sync.dma_start(out=xc,
                                  in_=xv_r[:, :, tci * CW:(tci + 1) * CW])
                xv_tiles[tci] = xc

            def v_proj_tile(ti, hp):
                # one head-pair's V slice [128 tok x 128 feat]: 8 accumulating
                # N=128 matmuls = ~427ns PE, so every head-pair injects only
                # its own V work (6.8us) instead of hp0 carrying a whole
                # dc-half (27.3us PE-bound pocket)
                xc = xv_tiles[ti // 4]
                ts2 = ti % 4
                ps = PSP[0].tile([128, CW], f32, tag="pp",
                                  name=f"pv_{ti}_{hp}")
                for kj in range(8):
                    mm(ps[:, 0:128], xc[:, kj, ts2 * 128:(ts2 + 1) * 128],
                       wv_sb[:, kj, hp * 128:(hp + 1) * 128],
                       kj == 0, kj == 7)
                nc.vector.tensor_copy(
                    out=Vp4[:, ti, 2 * hp:2 * hp + 2, 0:DK],
                    in_=ps[:, 0:128].rearrange("p (hh c) -> p hh c", c=DK))

            OT_tiles = {}

            # deferred normalization: (qc, hp, sbuf copy of [65,512] accum)
            pending_norm = []

            def flush_norm():
                while pending_norm:
                    qc, hp, oc = pending_norm.pop(0)
                    for hh in range(2):
                        # row 64 of oc = softmax denominator
                        rec = rec_pool.tile([1, 512], bf16, tag="rec",
                                            name=f"rec_{qc}_{hp}_{hh}")
                        with nc.allow_low_precision(
                                reason="softmax denom reciprocal, bf16 "
                                       "matches pipeline precision"):
                            nc.vector.reciprocal(out=rec,
                                                 in_=oc[hh][64:65, :])
                        # replicate reciprocal across 64 partitions via PE
                        ps_rep = PSP[0].tile([128, CW], f32, tag="pp",
                                              name=f"pr_{qc}_{hp}_{hh}")
                        mm(ps_rep[0:64, 0:512], ones_sb, rec, True, True)
                        recb = recb_pool.tile([64, 512], bf16, tag="recb",
                                              name=f"recb_{qc}_{hp}_{hh}")
                        nc.vector.tensor_copy(out=recb,
                                              in_=ps_rep[0:64, 0:512])
                        nc.vector.tensor_mul(
                            out=OT_tiles[qc][hh * 64:(hh + 1) * 64, hp, :],
                            in0=oc[hh][0:64, :], in1=recb)

            def qk_exp(qc, hp, kt):
                # both heads' scores^T into one 2-bank PSUM tile.
                # fp8 DoubleRow: contraction pairs (K_hi, Q8) + (K_lo, Q8)
                # -> (K_hi + K_lo)^T @ Q8 with K-side quantization error
                # corrected; charged 0.5 cycles/row by the PE.
                qsl = slice(qc * 512, (qc + 1) * 512)
                pss = PSS[0].tile([128, 1024], f32, tag="pss",
                                name=f"pss_{qc}_{hp}_{kt}")
                for hh in range(2):
                    pb = hh * 64
                    nc.tensor.matmul(
                        pss[:, hh * 512:(hh + 1) * 512],
                        lhsT=KT_sb[pb:pb + 64, hp, :,
                                   kt * 128:(kt + 1) * 128],
                        rhs=QT_sb[pb:pb + 64, hp, :, qsl],
                        start=True, stop=True,
                        perf_mode=mybir.MatmulPerfMode.DoubleRow)
                e = exp_pool.tile([128, 1024], bf16, tag="ex",
                                  name=f"ex_{qc}_{hp}_{kt}")
                nc.scalar.activation(out=e, in_=pss, func=AF.Exp,
                                     scale=SCALE)
                return e

            def attn_hp(qc, hp, inject=None, pre_e0=None, emit_next0=None):
                ps_o = [PSA[0].tile([128, 512], f32, tag="po",
                                   name=f"po_{qc}_{hp}_{i}")
                        for i in range(2)]

                def av(kt, e):
                    for hh in range(2):
                        h = 2 * hp + hh
                        mm(ps_o[hh][0:65, :],
                           Vp_sb[:, kt, 65 * h:65 * h + 65],
                           e[:, hh * 512:(hh + 1) * 512],
                           kt == 0, kt == 15)

                # QK/exp run one kt ahead of AV (and one slot across the
                # head-pair boundary via pre_e0/emit_next0) so ScalarE never
                # waits for the boundary accumulator handoff.
                e_prev = pre_e0
                kt0 = 0
                if pre_e0 is not None:
                    kt0 = 1
                    if inject is not None and inject.get(0) is not None:
                        inject[0]()
                for kt in range(kt0, 16):
                    if inject is not None and inject.get(kt) is not None:
                        inject[kt]()   # fine-grained PE fill-in (~1.7us max)
                    if kt == 2:
                        # previous pair's normalization, now off the
                        # critical path
                        flush_norm()
                    e = qk_exp(qc, hp, kt)
                    if e_prev is not None:
                        av(kt - 1, e_prev)
                    e_prev = e
                next_e = emit_next0() if emit_next0 is not None else None
                av(15, e_prev)
                # copy accumulators (incl. denominator row) to SBUF right
                # away: frees both PSUM slots for the next pair's AVs
                oc = []
                for hh in range(2):
                    o_sb = oc_pool.tile([65, 512], bf16, tag="oc",
                                        name=f"oc_{qc}_{hp}_{hh}")
                    nc.vector.tensor_copy(out=o_sb, in_=ps_o[hh][0:65, :])
                    oc.append(o_sb)
                pending_norm.append((qc, hp, oc))
                return next_e

            def wo_dj(qc, dj):
                qsl = slice(qc * 512, (qc + 1) * 512)
                ps_y = PSP[0].tile([128, CW], f32, tag="pp",
                                    name=f"py_{qc}_{dj}")
                for kj in range(8):
                    mm(ps_y[:, 0:512], wo_sb[:, kj, dj * 128:(dj + 1) * 128],
                       OT_tiles[qc][:, kj, :], kj == 0, kj == 7)
                yt = y_pool.tile([128, 512], bf16, tag="yt",
                                 name=f"yt_{qc}_{dj}")
                if qc == 1:
                    # tail: ACT is idle after attention ends; bias-add there
                    nc.scalar.activation(
                        out=yt, in_=ps_y[:, 0:512],
                        func=AF.Identity, bias=bo_sb[:, dj:dj + 1])
                else:
                    nc.vector.tensor_scalar_add(
                        out=yt, in0=ps_y[:, 0:512], scalar1=bo_sb[:, dj:dj + 1])
                nc.sync.dma_start(
                    out=yT[dj * 128:(dj + 1) * 128, qsl], in_=yt)

            OT_tiles[0] = ot_pool.tile([128, 8, 512], bf16, tag="OT",
                                       name="OT_0")
            nc.sync.dma_start(out=wq_sb, in_=wq_r)
            q_proj_dj(0, 0)   # attention head-pair 0 only needs Q dj0;
            q_proj_dj(0, 1)   # dj1 for the hp0->hp1 handoff. djs 2-7 are
            ps_proj.release()  # injected into hp0-5 below.
            PSP[0] = tc.alloc_tile_pool(name="ps_proj", bufs=2, space="PSUM")
            PSS[0] = tc.alloc_tile_pool(name="ps_s", bufs=2, space="PSUM")
            PSA[0] = tc.alloc_tile_pool(name="ps_av", bufs=2, space="PSUM")
            for tci in range(4):   # prefetch all xv chunks early
                xv_dma(tci)
            # Fine-grained PE fill-in inside qc0's ACT-bound head-pair loops:
            # one V-proj tile (~1.7us) or Q-proj dj block per kt slot, so the
            # QK->exp pipeline never starves for more than the 2-tile exp
            # buffer.
            # hp0 must produce V dc0 tiles at kt rate (its own AVs consume
            # them) and is PE-bound; spread the dc1 tiles over hp1-3 and the
            # qc1 Q-proj blocks over hp4-7 so the ACT-bound head-pairs stay
            # within their ~6us/hp PE fill-in budget.
            inj = {hp: {kt: (lambda ti=kt, h=hp: v_proj_tile(ti, h))
                        for kt in range(16)} for hp in range(8)}
            # Q-proj staggering: QT[0, dj=h+1] must exist before head-pair
            # h's handoff; QT[1, dj] likewise for qc1 (produced in qc0 hp6/7
            # and qc1 hp0-5)
            qsched = [(h, 0, h + 2) for h in range(6)] +                      [(6, 1, 0), (7, 1, 1)]
            for h, qc_, dj_ in qsched:
                inj[h][13] = (lambda a=inj[h].get(13), qq=qc_, dj=dj_:
                              (a() if a else None, q_proj_dj(qq, dj))[-1])
            inj_qc1 = {h: {13: (lambda dj=h + 2: q_proj_dj(1, dj))}
                       for h in range(6)}
            pre_e = None
            for hp in range(8):
                nxt = ((lambda h=hp + 1: qk_exp(0, h, 0)) if hp < 7
                       else (lambda: qk_exp(1, 0, 0)))
                pre_e = attn_hp(0, hp, inject=inj.get(hp), pre_e0=pre_e,
                                emit_next0=nxt)
            wo_sb = w_pool.tile([128, 8, D], bf16, tag="wbig", name="wo_sb")
            nc.sync.dma_start(out=wo_sb, in_=wo_r)
            OT_tiles[1] = ot_pool.tile([128, 8, 512], bf16, tag="OT",
                                       name="OT_1")
            for hp in range(8):
                nxt = (lambda h=hp + 1: qk_exp(1, h, 0)) if hp < 7 else None
                pre_e = attn_hp(1, hp, inject=inj_qc1.get(hp),
                                pre_e0=pre_e, emit_next0=nxt)
                if hp == 7:
                    # last pair's normalization before the final Wo block so
                    # its DVE chain hides under wo_dj(0,7)'s matmuls
                    flush_norm()
                wo_dj(0, hp)       # overlap qc0 output proj with qc1 attention
            for dj in range(8):
                wo_dj(1, dj)
            PSA[0].release()
            PSS[0].release()
            PSP[0].release()


def _xk8(kb):
    """[p, kj, j, t] fp8 pair layout: j0 = fp8(x), j1 = fp8(x/16)."""
    import ml_dtypes
    f8 = ml_dtypes.float8_e4m3
    xk = np.asarray(kb, np.float32).T            # [din, t]
    out = np.empty((128, 8, 2, xk.shape[1]), dtype=f8)
    out[:, :, 0, :] = xk.astype(f8).reshape(8, 128, -1).transpose(1, 0, 2)
    out[:, :, 1, :] = (xk / 16).astype(f8).reshape(8, 128, -1).transpose(1, 0, 2)
    return out


def _prep_inputs(query, key, value, Wq, bq, Wk, bk, Wv, bv, Wo, bo):
    import ml_dtypes
    bf = ml_dtypes.bfloat16

    query = np.asarray(query, np.float32)
    key = np.asarray(key, np.float32)
    value = np.asarray(value, np.float32)
    f8 = ml_dtypes.float8_e4m3
    wqT = np.ascontiguousarray(np.asarray(Wq, np.float32).T.astype(bf))
    WkT = np.asarray(Wk, np.float32).T           # [din, dout]
    wk_hi = WkT.astype(f8)
    wk_lo = ((WkT - wk_hi.astype(np.float32)) * 16).astype(f8)
    wk8 = np.empty((128, 8, 2, 1024), dtype=f8)  # [p, kj, j, dout]
    wk8[:, :, 0, :] = wk_hi.reshape(8, 128, 1024).transpose(1, 0, 2)
    wk8[:, :, 1, :] = wk_lo.reshape(8, 128, 1024).transpose(1, 0, 2)
    wvT = np.ascontiguousarray(np.asarray(Wv, np.float32).T.astype(bf))
    woT = np.ascontiguousarray(np.asarray(Wo, np.float32).T.astype(bf))
    bo_eff = np.asarray(bo, np.float32) + \
        np.asarray(Wo, np.float32) @ np.asarray(bv, np.float32)
    bq_t = np.ascontiguousarray(np.asarray(bq, np.float32).reshape(8, 128).T)
    bk_t = np.ascontiguousarray(np.asarray(bk, np.float32).reshape(8, 128).T)
    bo_t = np.ascontiguousarray(bo_eff.reshape(8, 128).T)

    in_maps = []
    for c in range(N_CORES):
        b, qh = c // 2, c % 2
        in_maps.append({
            "xqT": np.ascontiguousarray(
                query[b, qh * TQ:(qh + 1) * TQ, :].T.astype(bf)),
            "xk8": _xk8(key[b]),
            "xvT": np.ascontiguousarray(value[b].T.astype(bf)),
            "wqT": wqT, "wk8": wk8, "wvT": wvT, "woT": woT,
            "bq_in": bq_t, "bk_in": bk_t, "bo_in": bo_t,
        })
    return in_maps


def kernel(query, key, value, Wq, bq, Wk, bk, Wv, bv, Wo, bo):
    from concourse.bass_utils import run_bass_kernel_spmd

    if "nc" not in _CACHE:
        _CACHE["nc"] = _build_program()
    nc = _CACHE["nc"]

    in_maps = _prep_inputs(query, key, value, Wq, bq, Wk, bk, Wv, bv, Wo, bo)
    res = run_bass_kernel_spmd(nc, in_maps, list(range(N_CORES)))
    out = np.empty((B, S, D), np.float32)
    for c in range(N_CORES):
        b, qh = c // 2, c % 2
        out[b, qh * TQ:(qh + 1) * TQ, :] = \
            res.results[c]["yT"].T.astype(np.float32)
    return out

